# revision 1
# baseline (speedup 1.0000x reference)
"""Trainium2 Bass kernel for nn_BaconAdditionReasoner (segment_reduce).

Math (per row b of 1M):
  a = p1 @ minmax(W1); b = p2 @ minmax(W2)           # [10] each
  s_ij = min(a_i, b_j); one_minus = 1 - clip(s)       # [10,10]
  y_k  = 1 - prod_{i+j=k} one_minus_ij                # 19 anti-diag bins
  y    = y / (sum_k y_k + 1e-9)

Kernel formulation (avoids materializing min/clip and the mask matmul):
  alpha = p1 @ (1 - minmax(W1))  (rows of p1 sum to 1)  -> one_minus rows
  s_log_ij = max(ln(alpha_i), ln(beta_j))   [log is monotone; the
      reference clip at 1e-6/1-1e-6 never fires for this distribution]
  logP_k = sum over anti-diagonal (stride-9 slices of the flattened
      10x10; mirror bins k and 18-k fused into one paired reduce)
  y = (1 - exp(logP)) normalized by (19 + 1e-9 - sum exp(logP)).

Layout: batch rows on the 128 partitions, R rows per partition per
bigtile (2 warm-up tiles at R=48 for fast pipeline fill, then R=128),
rows contiguous in HBM per partition. The per-row 10x10 matmuls run on
the PE via 12-row-packed transposes (lhsT = transposed p-block, rhs =
kron(I_12, V)); Ln/Exp/copies on ACT; outer-max, paired reduces and
normalize on DVE.

Sharding: pure data parallel over 8 cores, 131072 rows each.
"""
import sys

if '/opt/trn_rl_repo' not in sys.path:
    sys.path.insert(0, '/opt/trn_rl_repo')

import numpy as np

B = 1048576
N_CORES = 8
RPC = B // N_CORES          # 131072 rows per core
P = 128                     # partitions
NT = 16                     # work units of 8192 rows (for bench scaling)

CNT = [min(k, 18 - k) + 1 for k in range(19)]
I0 = [max(0, k - 9) for k in range(19)]


def _groups_for(r):
    """r-slices per PE transpose group (12 rows of 10 -> K=120)."""
    g = [12] * (r // 12)
    if r % 12:
        g.append(r % 12)
    return g


def _schedule(nt):
    """Tile schedule: two small R=48 tiles first so the DVE phase starts
    early (short pipeline-fill), then R=128 tiles for low per-instruction
    overhead. Returns [(row0, R), ...] covering nt*8192 rows."""
    rows = nt * P * 64
    out, row0 = [], 0
    if rows >= P * 2 * 48 + P * 128:
        for _ in range(2):
            out.append((row0, 48)); row0 += P * 48
    while rows - row0 >= P * 128:
        out.append((row0, 128)); row0 += P * 128
    while rows - row0 > 0:
        r = (rows - row0) // P
        assert r > 0 and (rows - row0) % P == 0
        out.append((row0, r)); row0 += P * r
    return out

_CACHED = {}


def _build_nc(nt=NT, reps=1):
    import bass_rust as _br
    import concourse.mybir as mybir
    from concourse.bacc import Bacc
    from concourse.mybir import AluOpType
    from concourse.tile import TileContext

    F32 = mybir.dt.float32

    # Bacc (not Bass): its finalize() runs move_matmul_waits_to_ldweights +
    # generate_event_semaphores, required because walrus allows only one
    # sync wait on a self-loading fp32 Matmult.
    nc = Bacc()
    p1d = nc.dram_tensor("p1", [RPC, 10], F32, kind="ExternalInput")
    p2d = nc.dram_tensor("p2", [RPC, 10], F32, kind="ExternalInput")
    v1d = nc.dram_tensor("v1b", [120, 120], F32, kind="ExternalInput")
    v2d = nc.dram_tensor("v2b", [120, 120], F32, kind="ExternalInput")
    idd = nc.dram_tensor("ident", [128, 128], F32, kind="ExternalInput")
    yd = nc.dram_tensor("y", [RPC, 19], F32, kind="ExternalOutput")

    sched = _schedule(nt)

    with TileContext(nc) as tc:
        with (
            tc.tile_pool(name="const", bufs=1) as cpool,
            tc.tile_pool(name="io", bufs=3) as io,
            tc.tile_pool(name="ab", bufs=2) as abp,
            tc.tile_pool(name="pt", bufs=3) as ptp,
            tc.tile_pool(name="s", bufs=2) as sp,
            tc.tile_pool(name="small", bufs=3) as sm,
            tc.tile_pool(name="tp", bufs=4, space="PSUM") as tpp,
            tc.tile_pool(name="mm", bufs=4, space="PSUM") as mmp,
        ):
            v1t = cpool.tile([120, 120], F32)
            v2t = cpool.tile([120, 120], F32)
            idt = cpool.tile([128, 128], F32)
            nc.sync.dma_start(v1t[:], v1d[:])
            nc.sync.dma_start(v2t[:], v2d[:])
            nc.sync.dma_start(idt[:], idd[:])

            for row0, R in [s for _ in range(reps) for s in sched]:
                nrows = P * R
                p1v = p1d[row0:row0 + nrows, :].rearrange(
                    "(p r) c -> p (r c)", p=P)
                p2v = p2d[row0:row0 + nrows, :].rearrange(
                    "(p r) c -> p (r c)", p=P)
                yv = yd[row0:row0 + nrows, :].rearrange(
                    "(p r) k -> p (r k)", p=P)
                p1t = io.tile([P, R * 10], F32, tag="p1t")
                p2t = io.tile([P, R * 10], F32, tag="p2t")
                nc.sync.dma_start(p1t[:], p1v)
                nc.sync.dma_start(p2t[:], p2v)

                abt = abp.tile([P, R, 20], F32, tag="ab")
                r0 = 0
                for gs in _groups_for(R):
                    K = gs * 10
                    for src, vt, o in ((p1t, v1t, 0), (p2t, v2t, 10)):
                        tp = tpp.tile([K, 128], F32, tag="tp")
                        nc.tensor.transpose(
                            tp[:], src[:, r0 * 10:(r0 + gs) * 10], idt[:])
                        pt = ptp.tile([K, 128], F32, tag="pt")
                        nc.scalar.copy(pt[:], tp[:])
                        mm = mmp.tile([P, K], F32, tag="mm")
                        nc.tensor.matmul(mm[:], pt[:], vt[0:K, 0:K],
                                         start=True, stop=True)
                        # Ln fused into the PSUM->SBUF copy (Copy and Ln
                        # share activation-table sets, so no extra loads)
                        nc.scalar.activation(
                            abt[:, r0:r0 + gs, o:o + 10],
                            mm[:].rearrange("p (r c) -> p r c", c=10),
                            mybir.ActivationFunctionType.Ln)
                    r0 += gs

                lab = abt  # already log(alpha)|log(beta)

                # s_log[:, r, i, j] = max(la_i, lb_j). The reference's
                # clip to [1e-6, 1-1e-6] is omitted: alpha/beta = p @ V with
                # V minmax-normalized and p a probability row, so values sit
                # far inside (0.1, 0.9) and the clip never fires.
                st = sp.tile([P, R, 10, 10], F32, tag="s")
                lpt = sm.tile([P, R, 19], F32, tag="lp")
                # For the very first tile, emit the outer-max + reduces per
                # PE group so the DVE phase starts as soon as the first
                # 12-row group's logs land (shaves pipeline-fill); later
                # tiles use whole-tile ops for minimal instruction count.
                if row0 == 0:
                    subs, rr = [], 0
                    for gs_ in _groups_for(R):
                        subs.append((rr, gs_)); rr += gs_
                else:
                    subs = [(0, R)]
                for sr0, srn in subs:
                    sl = slice(sr0, sr0 + srn)
                    a_v = lab[:, sl, 0:10].unsqueeze(3).broadcast_to(
                        (P, srn, 10, 10))
                    b_v = lab[:, sl, 10:20].unsqueeze(2).broadcast_to(
                        (P, srn, 10, 10))
                    nc.vector.tensor_tensor(st[:, sl], a_v, b_v,
                                            AluOpType.max)
                    # anti-diagonal log-sums; mirror bins k and 18-k share a
                    # count -> one paired strided reduce:
                    # in  [P, (r), (pair=2, step 99-11k), (cnt, step 9)]
                    # out [P, (r), (pair=2, step 18-2k), 1]
                    s_flat = st[:, sl].rearrange("p r a b -> p r (a b)")
                    for k in range(10):
                        cnt = CNT[k]
                        if k == 9:
                            nc.vector.tensor_reduce(
                                lpt[:, sl, 9:10],
                                s_flat[:, :, 9:9 + 81 + 1:9],
                                axis=mybir.AxisListType.X, op=AluOpType.add)
                            continue
                        seg = (s_flat[:, :, k:k + 9 * (cnt - 1) + 1:9]
                               if cnt > 1 else s_flat[:, :, k:k + 1])
                        raw = seg.ap
                        raw.insert(2, [99 - 11 * k, 2])
                        seg2 = _br.AP(tensor=seg.tensor, offset=seg.offset,
                                      ap=raw)
                        outb = lpt[:, sl, k:k + 1]
                        raw_o = outb.ap
                        raw_o.insert(2, [18 - 2 * k, 2])
                        out2 = _br.AP(tensor=outb.tensor, offset=outb.offset,
                                      ap=raw_o)
                        nc.vector.tensor_reduce(
                            out2, seg2, axis=mybir.AxisListType.X,
                            op=AluOpType.add)

                # P = exp(logP), in place on lpt
                nc.scalar.activation(
                    lpt[:].rearrange("p r k -> p (r k)"),
                    lpt[:].rearrange("p r k -> p (r k)"),
                    mybir.ActivationFunctionType.Exp)
                # denom = 19 + 1e-9 - sum(P); r = 1/denom
                spt = sm.tile([P, R], F32, tag="S")
                nc.vector.tensor_reduce(spt[:], lpt[:],
                                        axis=mybir.AxisListType.X,
                                        op=AluOpType.add)
                nc.vector.tensor_scalar(spt[:], spt[:], -1.0, 19.0 + 1e-9,
                                        AluOpType.mult, AluOpType.add)
                rt = sm.tile([P, R], F32, tag="r")
                nc.vector.reciprocal(rt[:], spt[:])
                # u = 1 - P on ACT (in place), then y = u*r (in place)
                nc.scalar.activation(
                    lpt[:].rearrange("p r k -> p (r k)"),
                    lpt[:].rearrange("p r k -> p (r k)"),
                    mybir.ActivationFunctionType.Copy, bias=1.0, scale=-1.0)
                r_b = rt[:].unsqueeze(2).broadcast_to((P, R, 19))
                nc.vector.tensor_tensor(lpt[:], lpt[:], r_b, AluOpType.mult)
                nc.sync.dma_start(yv, lpt[:].rearrange("p r k -> p (r k)"))

    nc.finalize()
    return nc


def _host_consts(W1, W2):
    def mmn(W):
        W = W.astype(np.float32)
        lo = W.min(1, keepdims=True)
        hi = W.max(1, keepdims=True)
        return (W - lo) / (hi - lo + np.float32(1e-8))

    eye12 = np.eye(12, dtype=np.float32)
    v1b = np.kron(eye12, (np.float32(1.0) - mmn(W1))).astype(np.float32)
    v2b = np.kron(eye12, (np.float32(1.0) - mmn(W2))).astype(np.float32)
    ident = np.eye(128, dtype=np.float32)
    return v1b, v2b, ident


def kernel(p1, p2, W1, W2, mask=None, **_unused):
    from concourse.bass_utils import run_bass_kernel_spmd

    if 'nc' not in _CACHED:
        _CACHED['nc'] = _build_nc()
    nc = _CACHED['nc']

    v1b, v2b, ident = _host_consts(W1, W2)
    p1 = np.ascontiguousarray(p1, dtype=np.float32)
    p2 = np.ascontiguousarray(p2, dtype=np.float32)

    in_maps = []
    for c in range(N_CORES):
        sl = slice(c * RPC, (c + 1) * RPC)
        in_maps.append({
            "p1": p1[sl], "p2": p2[sl],
            "v1b": v1b, "v2b": v2b, "ident": ident,
        })
    res = run_bass_kernel_spmd(nc, in_maps, list(range(N_CORES)))
    out = np.concatenate([res.results[c]["y"] for c in range(N_CORES)], axis=0)
    return out.astype(np.float32)


if __name__ == "__main__":
    rng = np.random.default_rng(0)
    p1 = rng.random((B, 10), dtype=np.float32)
    p1 /= p1.sum(1, keepdims=True)
    p2 = rng.random((B, 10), dtype=np.float32)
    p2 /= p2.sum(1, keepdims=True)
    W1 = rng.random((10, 10), dtype=np.float32)
    W2 = rng.random((10, 10), dtype=np.float32)
    y = kernel(p1, p2, W1, W2)
    print("kernel ran, y shape", y.shape, "sum", float(y.sum()))



# revision 10
# speedup vs baseline: 1.3770x; 1.3770x over previous
"""Trainium2 Bass kernel for nn_BaconAdditionReasoner (segment_reduce).

Math (per row b of 1M):
  a = p1 @ minmax(W1); b = p2 @ minmax(W2)           # [10] each
  s_ij = min(a_i, b_j); one_minus = 1 - clip(s)       # [10,10]
  y_k  = 1 - prod_{i+j=k} one_minus_ij                # 19 anti-diag bins
  y    = y / (sum_k y_k + 1e-9)

Kernel formulation (linear space, no Ln/Exp round-trip):
  A = p1 @ (1 - minmax(W1)); B = p2 @ (1 - minmax(W2))   # = one_minus rows
  one_minus_ij = max(A_i, B_j)   [clip never fires: A,B in (0.11, 0.84)]
  P_k = prod over anti-diagonal; y = (1 - P) / (19 + 1e-9 - sum P)

Engine split per tile (R rows/partition, rows on 128 partitions):
  PE    : 12-row-packed transposes + matmuls vs kron(I_12, V)
  ACT   : most PSUM->SBUF copies (transpose spill, batched A|B copies),
          u = 1 - P
  DVE   : bf16 pair-packed max over mirror-bin pairs for bins 2..16
          (all strides +-1 -> 2x_1p double rate); fp32 edge bins
          0/1/17/18 (error amplification P/(1-P) up to 4.1 there,
          <=0.54 for mid bins, so bf16 is safe only for the latter);
          bf16 multiply-trees for rows [0, Rd); reciprocal
  Pool  : multiply-trees for rows [Rd, R) via per-bin 2-dim APs (Pool
          ucode implements add/sub/mult/copy only -- no max/min/PSUM),
          the 19-bin add-tree for sum P, and y = u * r

Sharding: pure data parallel over 8 cores, 131072 rows each.
"""
import sys

if '/opt/trn_rl_repo' not in sys.path:
    sys.path.insert(0, '/opt/trn_rl_repo')

import numpy as np

B = 1048576
N_CORES = 8
RPC = B // N_CORES          # 131072 rows per core
P = 128                     # partitions
NT = 16                     # work units of 8192 rows (bench scaling)

# mirror-pair layout for mid bins (k, 18-k), k=2..8 (cnt c=k+1), plus bin 9
MID_KS = [2, 3, 4, 5, 6, 7, 8]
PP_BASE = {}
_off = 0
for _k in MID_KS:
    PP_BASE[_k] = _off
    _off += 2 * (_k + 1)
PP_BASE[9] = _off
PP_W = _off + 10            # 94


def _groups_for(r):
    g = [12] * (r // 12)
    if r % 12:
        g.append(r % 12)
    return g


def _batches_for(r):
    """Split the 12-row transpose groups into batches of <=4 groups (<=480
    PSUM columns) for batched PSUM->SBUF copies."""
    gs = _groups_for(r)
    out, cur = [], []
    for g in gs:
        cur.append(g)
        if len(cur) == 4:
            out.append(cur)
            cur = []
    if cur:
        out.append(cur)
    return out


def _schedule(nt):
    rows = nt * P * 64
    out, row0 = [], 0
    if rows >= P * 2 * 48 + P * 128:
        for _ in range(2):
            out.append((row0, 48)); row0 += P * 48
    while rows - row0 >= P * 128:
        out.append((row0, 128)); row0 += P * 128
    while rows - row0 > 0:
        r = (rows - row0) // P
        assert r > 0 and (rows - row0) % P == 0
        out.append((row0, r)); row0 += P * r
    return out

_CACHED = {}


def _emit_tile(nc, pools, consts, p1d, p2d, yd, row0, R, Rd):
    import bass_rust as _br
    import concourse.mybir as mybir
    from concourse.mybir import AluOpType

    F32 = mybir.dt.float32
    BF16 = mybir.dt.bfloat16
    io, ptp, abf, abb, pps, stps, lptp, e4p, sm, tpp, mmp = pools
    v1t, v2t, idt = consts
    Rp = R - Rd

    def ap(tile_ap, off, dims):
        return _br.AP(tensor=tile_ap.tensor, offset=tile_ap.offset + off,
                      ap=[list(tile_ap.ap[0])] + [list(d) for d in dims])

    nrows = P * R
    p1v = p1d[row0:row0 + nrows, :].rearrange("(p r) c -> p (r c)", p=P)
    p2v = p2d[row0:row0 + nrows, :].rearrange("(p r) c -> p (r c)", p=P)
    yv = yd[row0:row0 + nrows, :].rearrange("(p r) k -> p (r k)", p=P)
    p1t = io.tile([P, R * 10], F32, tag="p1t")
    p2t = io.tile([P, R * 10], F32, tag="p2t")
    nc.sync.dma_start(p1t[:], p1v)
    nc.sync.dma_start(p2t[:], p2v)

    abt = abf.tile([P, R, 20], F32, tag="ab")
    r0 = 0
    cp_idx = 0
    for batch in _batches_for(R):
        brows = sum(batch)
        mma = mmp.tile([P, 480], F32, tag="mma")
        mmb = mmp.tile([P, 480], F32, tag="mmb")
        off = 0
        for gs in batch:
            K = gs * 10
            for src, vt, mm in ((p1t, v1t, mma), (p2t, v2t, mmb)):
                tp = tpp.tile([K, 128], F32, tag="tp")
                nc.tensor.transpose(
                    tp[:], src[:, (r0 + off // 10) * 10:
                               (r0 + off // 10 + gs) * 10], idt[:])
                pt = ptp.tile([K, 128], F32, tag="pt")
                # PSUM->SBUF copies alternate ACT/DVE (GPSIMD cannot
                # access PSUM on hardware)
                if cp_idx % 4 != 3:
                    nc.scalar.copy(pt[:], tp[:])
                else:
                    nc.vector.tensor_copy(pt[:], tp[:])
                cp_idx += 1
                nc.tensor.matmul(mm[:, off:off + K], pt[:], vt[0:K, 0:K],
                                 start=True, stop=True)
            off += K
        # batched PSUM->SBUF copies: A and B halves of ab
        for mm, col in ((mma, 0), (mmb, 10)):
            dst = ap(abt[:], r0 * 20 + col, [[20, brows], [1, 10]])
            nc.scalar.copy(
                dst, mm[:, 0:brows * 10].rearrange("p (r c) -> p r c", c=10))
        r0 += brows

    lpt = lptp.tile([P, R, 19], F32, tag="lpt")

    # ---- DVE: bf16 pair-packed max for mid bins, fp32 edges (all rows) ----
    abb_t = abb.tile([P, R, 20], BF16, tag="abb")
    nc.vector.tensor_copy(abb_t[:], abt[:])
    pp = pps.tile([P, R, PP_W], BF16, tag="pp")
    A, Bo = 0, 10
    for k in MID_KS:
        c = k + 1
        base = PP_BASE[k]
        nc.vector.tensor_tensor(
            ap(pp[:], base, [[PP_W, R], [c, 2], [1, c]]),
            ap(abb_t[:], A, [[20, R], [9 - k, 2], [1, c]]),
            ap(abb_t[:], Bo + k, [[20, R], [9 - k, 2], [-1, c]]),
            AluOpType.max)
    nc.vector.tensor_tensor(
        ap(pp[:], PP_BASE[9], [[PP_W, R], [1, 10]]),
        ap(abb_t[:], A, [[20, R], [1, 10]]),
        ap(abb_t[:], Bo + 9, [[20, R], [-1, 10]]),
        AluOpType.max)
    # multiply-trees per pair: rows [0, Rd) on DVE (3-dim APs, bf16 2x),
    # rows [Rd, R) on Pool (per-bin 2-dim APs; Pool ucode has mult but
    # not max/min). Final level writes fp32 into lpt.
    for k in MID_KS + [9]:
        c = k + 1 if k != 9 else 10
        base = PP_BASE[k]
        pdims = [[c, 2]] if k != 9 else []
        c0 = c
        while c0 > 2:
            fl, ce = c0 // 2, c0 - c0 // 2
            nc.vector.tensor_tensor(
                ap(pp[:], base, [[PP_W, Rd]] + pdims + [[1, fl]]),
                ap(pp[:], base, [[PP_W, Rd]] + pdims + [[1, fl]]),
                ap(pp[:], base + ce, [[PP_W, Rd]] + pdims + [[1, fl]]),
                AluOpType.mult)
            c0 = ce
        odims = [[18 - 2 * k, 2]] if k != 9 else []
        nc.vector.tensor_tensor(
            ap(lpt[:], k, [[19, Rd]] + odims + [[1, 1]]),
            ap(pp[:], base, [[PP_W, Rd]] + pdims + [[1, 1]]),
            ap(pp[:], base + 1, [[PP_W, Rd]] + pdims + [[1, 1]]),
            AluOpType.mult)
        if Rp > 0:
            for psel, kk in (((0, k), (1, 18 - k)) if k != 9 else ((0, 9),)):
                pb = Rd * PP_W + base + psel * c
                c0 = c
                while c0 > 2:
                    fl, ce = c0 // 2, c0 - c0 // 2
                    nc.gpsimd.tensor_tensor(
                        ap(pp[:], pb, [[PP_W, Rp], [1, fl]]),
                        ap(pp[:], pb, [[PP_W, Rp], [1, fl]]),
                        ap(pp[:], pb + ce, [[PP_W, Rp], [1, fl]]),
                        AluOpType.mult)
                    c0 = ce
                nc.gpsimd.tensor_tensor(
                    ap(lpt[:], Rd * 19 + kk, [[19, Rp], [1, 1]]),
                    ap(pp[:], pb, [[PP_W, Rp], [1, 1]]),
                    ap(pp[:], pb + 1, [[PP_W, Rp], [1, 1]]),
                    AluOpType.mult)
    # edge bins 0/18 (single element) and 1/17 (two elements), fp32
    nc.vector.tensor_tensor(
        ap(lpt[:], 0, [[19, R], [18, 2]]),
        ap(abt[:], 0, [[20, R], [9, 2]]),
        ap(abt[:], 10, [[20, R], [9, 2]]),
        AluOpType.max)
    e4 = e4p.tile([P, R, 2, 2], F32, tag="e4")
    nc.vector.tensor_tensor(
        e4[:],
        ap(abt[:], 0, [[20, R], [8, 2], [1, 2]]),
        ap(abt[:], 11, [[20, R], [8, 2], [-1, 2]]),
        AluOpType.max)
    nc.vector.tensor_tensor(
        ap(lpt[:], 1, [[19, R], [16, 2]]),
        e4[:, :, :, 0], e4[:, :, :, 1], AluOpType.mult)

    # ---- normalize: y = (1 - P) / (19 + 1e-9 - sum P) ----
    # sum P as an add-tree on Pool (19 = 9+9+1: pairs then halvings)
    st_ = sm.tile([P, R], F32, tag="S")
    sacc = stps.tile([P, R, 9], F32, tag="sacc")
    nc.gpsimd.tensor_tensor(sacc[:], lpt[:, :, 0:9], lpt[:, :, 10:19],
                            AluOpType.add)
    nc.gpsimd.tensor_tensor(sacc[:, :, 0:4], sacc[:, :, 0:4],
                            sacc[:, :, 5:9], AluOpType.add)
    nc.gpsimd.tensor_tensor(sacc[:, :, 0:2], sacc[:, :, 0:2],
                            sacc[:, :, 2:4], AluOpType.add)
    nc.gpsimd.tensor_tensor(sacc[:, :, 0:1], sacc[:, :, 0:1],
                            sacc[:, :, 1:2], AluOpType.add)
    nc.gpsimd.tensor_tensor(sacc[:, :, 4:5], sacc[:, :, 4:5],
                            lpt[:, :, 9:10], AluOpType.add)
    nc.gpsimd.tensor_tensor(st_[:].unsqueeze(2), sacc[:, :, 0:1],
                            sacc[:, :, 4:5], AluOpType.add)
    nc.vector.tensor_scalar(st_[:], st_[:], -1.0, 19.0 + 1e-9,
                            AluOpType.mult, AluOpType.add)
    rt = sm.tile([P, R], F32, tag="r")
    nc.vector.reciprocal(rt[:], st_[:])
    # u = 1 - P on ACT (in place)
    nc.scalar.activation(
        lpt[:].rearrange("p r k -> p (r k)"),
        lpt[:].rearrange("p r k -> p (r k)"),
        mybir.ActivationFunctionType.Copy, bias=1.0, scale=-1.0)
    # y = u * r on Pool (in place), r broadcast along k
    nc.gpsimd.tensor_tensor(
        lpt[:], lpt[:],
        rt[:].unsqueeze(2).broadcast_to((P, R, 19)),
        AluOpType.mult)
    nc.sync.dma_start(yv, lpt[:].rearrange("p r k -> p (r k)"))


def _build_core(rows_total, sched, rd_map):
    import concourse.mybir as mybir
    from concourse.bacc import Bacc
    from concourse.tile import TileContext

    F32 = mybir.dt.float32

    nc = Bacc()
    p1d = nc.dram_tensor("p1", [rows_total, 10], F32, kind="ExternalInput")
    p2d = nc.dram_tensor("p2", [rows_total, 10], F32, kind="ExternalInput")
    v1d = nc.dram_tensor("v1b", [120, 120], F32, kind="ExternalInput")
    v2d = nc.dram_tensor("v2b", [120, 120], F32, kind="ExternalInput")
    idd = nc.dram_tensor("ident", [128, 128], F32, kind="ExternalInput")
    yd = nc.dram_tensor("y", [rows_total, 19], F32, kind="ExternalOutput")

    with TileContext(nc) as tc:
        with (
            tc.tile_pool(name="const", bufs=1) as cpool,
            tc.tile_pool(name="io", bufs=3) as io,
            tc.tile_pool(name="pt", bufs=3) as ptp,
            tc.tile_pool(name="abf", bufs=2) as abf,
            tc.tile_pool(name="abb", bufs=2) as abb,
            tc.tile_pool(name="pps", bufs=2) as pps,
            tc.tile_pool(name="stp", bufs=2) as stps,
            tc.tile_pool(name="lpt", bufs=2) as lptp,
            tc.tile_pool(name="e4", bufs=2) as e4p,
            tc.tile_pool(name="sm", bufs=2) as sm,
            tc.tile_pool(name="tp", bufs=3, space="PSUM") as tpp,
            tc.tile_pool(name="mm", bufs=2, space="PSUM") as mmp,
        ):
            v1t = cpool.tile([120, 120], F32)
            v2t = cpool.tile([120, 120], F32)
            idt = cpool.tile([128, 128], F32)
            nc.sync.dma_start(v1t[:], v1d[:])
            nc.sync.dma_start(v2t[:], v2d[:])
            nc.sync.dma_start(idt[:], idd[:])
            pools = (io, ptp, abf, abb, pps, stps, lptp, e4p, sm, tpp, mmp)
            for row0, R in sched:
                _emit_tile(nc, pools, (v1t, v2t, idt), p1d, p2d, yd,
                           row0, R, rd_map[R])

    nc.finalize()
    return nc


RD_MAP = {128: 28, 48: 10, 32: 8, 16: 4, 8: 4}


def _build_nc(nt=NT, reps=1):
    sched = [s for _ in range(reps) for s in _schedule(nt)]
    return _build_core(RPC, sched, RD_MAP)


def _host_consts(W1, W2):
    def mmn(W):
        W = W.astype(np.float32)
        lo = W.min(1, keepdims=True)
        hi = W.max(1, keepdims=True)
        return (W - lo) / (hi - lo + np.float32(1e-8))

    eye12 = np.eye(12, dtype=np.float32)
    v1b = np.kron(eye12, (np.float32(1.0) - mmn(W1))).astype(np.float32)
    v2b = np.kron(eye12, (np.float32(1.0) - mmn(W2))).astype(np.float32)
    ident = np.eye(128, dtype=np.float32)
    return v1b, v2b, ident


def kernel(p1, p2, W1, W2, mask=None, **_unused):
    from concourse.bass_utils import run_bass_kernel_spmd

    if 'nc' not in _CACHED:
        _CACHED['nc'] = _build_nc()
    nc = _CACHED['nc']

    v1b, v2b, ident = _host_consts(W1, W2)
    p1 = np.ascontiguousarray(p1, dtype=np.float32)
    p2 = np.ascontiguousarray(p2, dtype=np.float32)

    in_maps = []
    for c in range(N_CORES):
        sl = slice(c * RPC, (c + 1) * RPC)
        in_maps.append({
            "p1": p1[sl], "p2": p2[sl],
            "v1b": v1b, "v2b": v2b, "ident": ident,
        })
    res = run_bass_kernel_spmd(nc, in_maps, list(range(N_CORES)))
    out = np.concatenate([res.results[c]["y"] for c in range(N_CORES)], axis=0)
    return out.astype(np.float32)


def _numpy_ref(p1, p2, W1, W2):
    def mmn(W):
        lo = W.min(1, keepdims=True); hi = W.max(1, keepdims=True)
        return (W - lo) / (hi - lo + np.float32(1e-8))
    A = p1 @ (1.0 - mmn(W1))
    Bv = p2 @ (1.0 - mmn(W2))
    st = np.maximum(A[:, :, None], Bv[:, None, :])
    Pk = np.ones((p1.shape[0], 19), np.float64)
    for i in range(10):
        for j in range(10):
            Pk[:, i + j] *= st[:, i, j]
    y = 1.0 - Pk
    return (y / (y.sum(1, keepdims=True) + 1e-9)).astype(np.float32)


def _simcheck():
    """CoreSim-exec validation on a small schedule."""
    from concourse.bass_interp import CoreSim
    rows = P * 16
    nc = _build_core(rows, [(0, 8), (P * 8, 8)], RD_MAP)
    rng = np.random.default_rng(1)
    p1 = rng.random((rows, 10), dtype=np.float32)
    p1 /= p1.sum(1, keepdims=True)
    p2 = rng.random((rows, 10), dtype=np.float32)
    p2 /= p2.sum(1, keepdims=True)
    W1 = rng.random((10, 10), dtype=np.float32)
    W2 = rng.random((10, 10), dtype=np.float32)
    v1b, v2b, ident = _host_consts(W1, W2)

    vals = {"p1": p1, "p2": p2, "v1b": v1b, "v2b": v2b, "ident": ident}
    bufs = {}
    for alloc in nc.m.functions[0].allocations:
        if hasattr(alloc, 'memorylocations') and alloc.memorylocations:
            for mem in alloc.memorylocations:
                if mem.name in vals:
                    a = np.ascontiguousarray(vals[mem.name])
                    bufs[mem.name] = a.view(np.uint8).reshape(tuple(mem.dims))
                elif mem.name == "y":
                    bufs["y"] = np.zeros(tuple(mem.dims), np.uint8)
    sim = CoreSim(nc, preallocated_bufs=bufs)
    sim.simulate()
    y = bufs["y"].view(np.float32).reshape(rows, 19)
    ref = _numpy_ref(p1, p2, W1, W2)
    rel = np.abs(y - ref) / np.maximum(np.abs(ref), 1e-12)
    print(f"simcheck: rel err max {rel.max():.3e}  sim time {sim.time:.0f} ns")
    assert rel.max() < 2e-2, "simcheck FAILED"
    print("simcheck PASSED")


if __name__ == "__main__":
    if "--simcheck" in sys.argv:
        _simcheck()
    else:
        rng = np.random.default_rng(0)
        p1 = rng.random((B, 10), dtype=np.float32)
        p1 /= p1.sum(1, keepdims=True)
        p2 = rng.random((B, 10), dtype=np.float32)
        p2 /= p2.sum(1, keepdims=True)
        W1 = rng.random((10, 10), dtype=np.float32)
        W2 = rng.random((10, 10), dtype=np.float32)
        y = kernel(p1, p2, W1, W2)
        print("kernel ran, y shape", y.shape, "sum", float(y.sum()))


# revision 21
# speedup vs baseline: 2.4971x; 1.8134x over previous
"""Trainium2 Bass kernel for nn_BaconAdditionReasoner (segment_reduce).

Math (per row b of 1M):
  a = p1 @ minmax(W1); b = p2 @ minmax(W2)           # [10] each
  s_ij = min(a_i, b_j); one_minus = 1 - clip(s)       # [10,10]
  y_k  = 1 - prod_{i+j=k} one_minus_ij                # 19 anti-diag bins
  y    = y / (sum_k y_k + 1e-9)

Kernel formulation (linear space, no Ln/Exp round-trip):
  A = p1 @ (1 - minmax(W1)); B = p2 @ (1 - minmax(W2))   # = one_minus rows
  one_minus_ij = max(A_i, B_j)   [clip never fires: A,B in (0.11, 0.84)]
  P_k = prod over anti-diagonal; y = (1 - P) / (19 + 1e-9 - sum P)

Engine split per tile (R rows/partition, rows on 128 partitions):
  PE    : 12-row-packed transposes + matmuls vs kron(I_12, V)
  ACT   : PSUM->SBUF copies (4-group-batched transpose spill and A|B
          copies), u = 1 - P
  DVE   : bf16 pair-packed max over mirror-bin pairs for bins 2..16
          (all strides +-1 -> 2x_1p double rate); fp32 edge bins
          0/1/17/18 (error amplification P/(1-P) is up to 4.1 there,
          <=0.54 for mid bins, so bf16 is only safe for the latter);
          bf16 multiply-trees for rows [0, Rd); reciprocal
  Pool  : multiply-trees for rows [Rd, R) via per-bin 2-dim APs (Pool
          ucode implements add/sub/mult/copy only -- no max/min/PSUM),
          the 19-bin add-tree for sum P, and y = u * r

Tiles are emitted software-pipelined (frontend of tile i+1 before the
backend of tile i, input DMAs two tiles ahead) so the in-order engine
queues interleave ready work instead of stalling on cross-engine waits.

Sharding: pure data parallel over 8 cores, 131072 rows each.
"""
import sys

if '/opt/trn_rl_repo' not in sys.path:
    sys.path.insert(0, '/opt/trn_rl_repo')

import numpy as np

B = 1048576
N_CORES = 8
RPC = B // N_CORES          # 131072 rows per core
P = 128                     # partitions
NT = 16                     # work units of 8192 rows (bench scaling)

# mirror-pair layout for mid bins (k, 18-k), k=2..8 (cnt c=k+1), plus bin 9
MID_KS = [2, 3, 4, 5, 6, 7, 8]
PP_BASE = {}
_off = 0
for _k in MID_KS:
    PP_BASE[_k] = _off
    _off += 2 * (_k + 1)
PP_BASE[9] = _off
PP_W = _off + 10            # 94


def _groups_for(r):
    g = [12] * (r // 12)
    if r % 12:
        g.append(r % 12)
    return g


def _batches_for(r):
    """<=4 transpose groups (<=480 PSUM columns) per PSUM tile / copy."""
    gs = _groups_for(r)
    out, cur = [], []
    for g in gs:
        cur.append(g)
        if len(cur) == 4:
            out.append(cur)
            cur = []
    if cur:
        out.append(cur)
    return out


def _schedule(nt):
    rows = nt * P * 64
    out, row0 = [], 0
    if rows >= P * 2 * 48 + P * 128:
        for _ in range(2):
            out.append((row0, 48)); row0 += P * 48
    while rows - row0 >= P * 128:
        out.append((row0, 128)); row0 += P * 128
    while rows - row0 > 0:
        r = (rows - row0) // P
        assert r > 0 and (rows - row0) % P == 0
        out.append((row0, r)); row0 += P * r
    return out

_CACHED = {}


def _mkap(br, tile_ap, off, dims):
    return br.AP(tensor=tile_ap.tensor, offset=tile_ap.offset + off,
                 ap=[list(tile_ap.ap[0])] + [list(d) for d in dims])


def _emit_loads(nc, io, p1d, p2d, row0, R):
    import concourse.mybir as mybir
    F32 = mybir.dt.float32
    nrows = P * R
    p1v = p1d[row0:row0 + nrows, :].rearrange("(p r) c -> p (r c)", p=P)
    p2v = p2d[row0:row0 + nrows, :].rearrange("(p r) c -> p (r c)", p=P)
    p1t = io.tile([P, R * 10], F32, tag="p1t")
    p2t = io.tile([P, R * 10], F32, tag="p2t")
    nc.sync.dma_start(p1t[:], p1v)
    nc.sync.dma_start(p2t[:], p2v)
    return p1t, p2t


def _emit_front(nc, pools, consts, R, loaded):
    """Transposes, matmuls, PSUM->SBUF copies.

    Produces abb [P, R, 20] bf16 (A | B rows for the pair-packed max) and
    abe [P, R, 8] fp32 (columns {0,1,8,9} of A then B, for the fp32 edge
    bins)."""
    import bass_rust as _br
    import concourse.mybir as mybir

    F32 = mybir.dt.float32
    BF16 = mybir.dt.bfloat16
    io, ptp, abf, abb, pps, lptp, e4p, sm, tpp, mmp = pools
    v1t, v2t, idt = consts
    p1t, p2t = loaded

    abb_t = abb.tile([P, R, 20], BF16, tag="abb")
    abe = abf.tile([P, R, 8], F32, tag="abe")
    r0 = 0
    for batch in _batches_for(R):
        brows = sum(batch)
        bcols = brows * 10
        mma = mmp.tile([P, 480], F32, tag="mma")
        mmb = mmp.tile([P, 480], F32, tag="mmb")
        tpa = tpp.tile([120, 512], F32, tag="tpa")
        tpb = tpp.tile([120, 512], F32, tag="tpb")
        pta = ptp.tile([120, 512], F32, tag="pta")
        ptb = ptp.tile([120, 512], F32, tag="ptb")
        for src, vt, mm, tp, pt in ((p1t, v1t, mma, tpa, pta),
                                    (p2t, v2t, mmb, tpb, ptb)):
            off = 0
            full_k = 0
            for gi, gs in enumerate(batch):
                K = gs * 10
                nc.tensor.transpose(
                    tp[0:K, gi * 128:(gi + 1) * 128],
                    src[:, (r0 * 10 + off):(r0 * 10 + off + K)], idt[:])
                off += K
                if K == 120:
                    full_k += 1
            # batched PSUM->SBUF copy: full 120-row groups in one shot,
            # the ragged tail group (if any) separately
            if full_k:
                nc.scalar.copy(pt[:, 0:full_k * 128], tp[:, 0:full_k * 128])
            if full_k < len(batch):
                K = batch[-1] * 10
                nc.scalar.copy(pt[0:K, full_k * 128:full_k * 128 + 128],
                               tp[0:K, full_k * 128:full_k * 128 + 128])
            off = 0
            for gi, gs in enumerate(batch):
                K = gs * 10
                nc.tensor.matmul(mm[:, off:off + K],
                                 pt[0:K, gi * 128:(gi + 1) * 128],
                                 vt[0:K, 0:K], start=True, stop=True)
                off += K
        for mm, col in ((mma, 0), (mmb, 10)):
            dst = _mkap(_br, abb_t[:], r0 * 20 + col, [[20, brows], [1, 10]])
            nc.scalar.copy(
                dst, mm[:, 0:bcols].rearrange("p (r c) -> p r c", c=10))
        for mm, col in ((mma, 0), (mmb, 4)):
            dst = _mkap(_br, abe[:], r0 * 8 + col,
                        [[8, brows], [2, 2], [1, 2]])
            src = _mkap(_br, mm[:, 0:bcols], 0, [[10, brows], [8, 2], [1, 2]])
            nc.scalar.copy(dst, src)
        r0 += brows
    return abb_t, abe


def _emit_back(nc, pools, front, yv, R, Rd):
    """Max, multiply-trees, normalize, store."""
    import bass_rust as _br
    import concourse.mybir as mybir
    from concourse.mybir import AluOpType

    F32 = mybir.dt.float32
    BF16 = mybir.dt.bfloat16
    io, ptp, abf, abb, pps, lptp, e4p, sm, tpp, mmp = pools
    abb_t, abe = front
    Rp = R - Rd

    def ap(tile_ap, off, dims):
        return _mkap(_br, tile_ap, off, dims)

    lpt = lptp.tile([P, R, 19], F32, tag="lpt")
    pp = pps.tile([P, R, PP_W], BF16, tag="pp")
    A, Bo = 0, 10
    for k in MID_KS:
        c = k + 1
        base = PP_BASE[k]
        nc.vector.tensor_tensor(
            ap(pp[:], base, [[PP_W, R], [c, 2], [1, c]]),
            ap(abb_t[:], A, [[20, R], [9 - k, 2], [1, c]]),
            ap(abb_t[:], Bo + k, [[20, R], [9 - k, 2], [-1, c]]),
            AluOpType.max)
    nc.vector.tensor_tensor(
        ap(pp[:], PP_BASE[9], [[PP_W, R], [1, 10]]),
        ap(abb_t[:], A, [[20, R], [1, 10]]),
        ap(abb_t[:], Bo + 9, [[20, R], [-1, 10]]),
        AluOpType.max)
    # edge bins 0/18 and 1/17 in fp32 (abe cols: A0 A1 A8 A9 B0 B1 B8 B9)
    nc.vector.tensor_tensor(
        ap(lpt[:], 0, [[19, R], [18, 2]]),
        ap(abe[:], 0, [[8, R], [3, 2]]),
        ap(abe[:], 4, [[8, R], [3, 2]]),
        AluOpType.max)
    e4 = e4p.tile([P, R, 2, 2], F32, tag="e4")
    nc.vector.tensor_tensor(
        e4[:],
        ap(abe[:], 0, [[8, R], [2, 2], [1, 2]]),
        ap(abe[:], 5, [[8, R], [2, 2], [-1, 2]]),
        AluOpType.max)
    nc.gpsimd.tensor_tensor(
        ap(lpt[:], 1, [[19, R], [16, 2]]),
        e4[:, :, :, 0], e4[:, :, :, 1], AluOpType.mult)
    # multiply-trees: rows [0, Rd) on DVE (bf16 2x pair APs), rows
    # [Rd, R) on Pool (per-bin 2-dim APs). Final level writes fp32 lpt.
    for k in MID_KS + [9]:
        c = k + 1 if k != 9 else 10
        base = PP_BASE[k]
        pdims = [[c, 2]] if k != 9 else []
        if Rd > 0:
            c0 = c
            while c0 > 2:
                fl, ce = c0 // 2, c0 - c0 // 2
                nc.vector.tensor_tensor(
                    ap(pp[:], base, [[PP_W, Rd]] + pdims + [[1, fl]]),
                    ap(pp[:], base, [[PP_W, Rd]] + pdims + [[1, fl]]),
                    ap(pp[:], base + ce, [[PP_W, Rd]] + pdims + [[1, fl]]),
                    AluOpType.mult)
                c0 = ce
            odims = [[18 - 2 * k, 2]] if k != 9 else []
            nc.vector.tensor_tensor(
                ap(lpt[:], k, [[19, Rd]] + odims + [[1, 1]]),
                ap(pp[:], base, [[PP_W, Rd]] + pdims + [[1, 1]]),
                ap(pp[:], base + 1, [[PP_W, Rd]] + pdims + [[1, 1]]),
                AluOpType.mult)
        if Rp > 0:
            for psel, kk in (((0, k), (1, 18 - k)) if k != 9 else ((0, 9),)):
                pb = Rd * PP_W + base + psel * c
                c0 = c
                while c0 > 2:
                    fl, ce = c0 // 2, c0 - c0 // 2
                    nc.gpsimd.tensor_tensor(
                        ap(pp[:], pb, [[PP_W, Rp], [1, fl]]),
                        ap(pp[:], pb, [[PP_W, Rp], [1, fl]]),
                        ap(pp[:], pb + ce, [[PP_W, Rp], [1, fl]]),
                        AluOpType.mult)
                    c0 = ce
                nc.gpsimd.tensor_tensor(
                    ap(lpt[:], Rd * 19 + kk, [[19, Rp], [1, 1]]),
                    ap(pp[:], pb, [[PP_W, Rp], [1, 1]]),
                    ap(pp[:], pb + 1, [[PP_W, Rp], [1, 1]]),
                    AluOpType.mult)

    # normalize: y = (1 - P) / (19 + 1e-9 - sum P); sum P as Pool add-tree
    st_ = sm.tile([P, R], F32, tag="S")
    sacc = sm.tile([P, R, 9], F32, tag="sacc")
    nc.gpsimd.tensor_tensor(sacc[:], lpt[:, :, 0:9], lpt[:, :, 10:19],
                            AluOpType.add)
    nc.gpsimd.tensor_tensor(sacc[:, :, 0:4], sacc[:, :, 0:4],
                            sacc[:, :, 5:9], AluOpType.add)
    nc.gpsimd.tensor_tensor(sacc[:, :, 0:2], sacc[:, :, 0:2],
                            sacc[:, :, 2:4], AluOpType.add)
    nc.gpsimd.tensor_tensor(sacc[:, :, 0:1], sacc[:, :, 0:1],
                            sacc[:, :, 1:2], AluOpType.add)
    nc.gpsimd.tensor_tensor(sacc[:, :, 4:5], sacc[:, :, 4:5],
                            lpt[:, :, 9:10], AluOpType.add)
    nc.gpsimd.tensor_tensor(st_[:].unsqueeze(2), sacc[:, :, 0:1],
                            sacc[:, :, 4:5], AluOpType.add)
    nc.vector.tensor_scalar(st_[:], st_[:], -1.0, 19.0 + 1e-9,
                            AluOpType.mult, AluOpType.add)
    rt = sm.tile([P, R], F32, tag="r")
    nc.vector.reciprocal(rt[:], st_[:])
    nc.scalar.activation(
        lpt[:].rearrange("p r k -> p (r k)"),
        lpt[:].rearrange("p r k -> p (r k)"),
        mybir.ActivationFunctionType.Copy, bias=1.0, scale=-1.0)
    nc.gpsimd.tensor_tensor(
        lpt[:], lpt[:],
        rt[:].unsqueeze(2).broadcast_to((P, R, 19)),
        AluOpType.mult)
    nc.sync.dma_start(yv, lpt[:].rearrange("p r k -> p (r k)"))


def _build_core(rows_total, sched, rd_map):
    import concourse.mybir as mybir
    from concourse.bacc import Bacc
    from concourse.tile import TileContext

    F32 = mybir.dt.float32

    nc = Bacc()
    p1d = nc.dram_tensor("p1", [rows_total, 10], F32, kind="ExternalInput")
    p2d = nc.dram_tensor("p2", [rows_total, 10], F32, kind="ExternalInput")
    v1d = nc.dram_tensor("v1b", [120, 120], F32, kind="ExternalInput")
    v2d = nc.dram_tensor("v2b", [120, 120], F32, kind="ExternalInput")
    idd = nc.dram_tensor("ident", [128, 128], F32, kind="ExternalInput")
    yd = nc.dram_tensor("y", [rows_total, 19], F32, kind="ExternalOutput")

    with TileContext(nc) as tc:
        with (
            tc.tile_pool(name="const", bufs=1) as cpool,
            tc.tile_pool(name="io", bufs=4) as io,
            tc.tile_pool(name="pt", bufs=2) as ptp,
            tc.tile_pool(name="abf", bufs=3) as abf,
            tc.tile_pool(name="abb", bufs=3) as abb,
            tc.tile_pool(name="pps", bufs=2) as pps,
            tc.tile_pool(name="lpt", bufs=3) as lptp,
            tc.tile_pool(name="e4", bufs=2) as e4p,
            tc.tile_pool(name="sm", bufs=2) as sm,
            tc.tile_pool(name="tp", bufs=2, space="PSUM") as tpp,
            tc.tile_pool(name="mm", bufs=2, space="PSUM") as mmp,
        ):
            v1t = cpool.tile([120, 120], F32)
            v2t = cpool.tile([120, 120], F32)
            idt = cpool.tile([128, 128], F32)
            nc.sync.dma_start(v1t[:], v1d[:])
            nc.sync.dma_start(v2t[:], v2d[:])
            nc.sync.dma_start(idt[:], idd[:])
            pools = (io, ptp, abf, abb, pps, lptp, e4p, sm, tpp, mmp)
            consts = (v1t, v2t, idt)

            n = len(sched)
            loads, fronts = {}, {}
            for i in range(min(2, n)):
                loads[i] = _emit_loads(nc, io, p1d, p2d, *sched[i])
            if n:
                fronts[0] = _emit_front(nc, pools, consts, sched[0][1],
                                        loads.pop(0))
            for i in range(n):
                if i + 2 < n:
                    loads[i + 2] = _emit_loads(nc, io, p1d, p2d,
                                               *sched[i + 2])
                if i + 1 < n:
                    fronts[i + 1] = _emit_front(nc, pools, consts,
                                                sched[i + 1][1],
                                                loads.pop(i + 1))
                row0, R = sched[i]
                nrows = P * R
                yv = yd[row0:row0 + nrows, :].rearrange(
                    "(p r) k -> p (r k)", p=P)
                _emit_back(nc, pools, fronts.pop(i), yv, R, rd_map[R])

    nc.finalize()
    return nc


RD_MAP = {128: 24, 48: 10, 32: 6, 16: 3, 8: 2}


def _build_nc(nt=NT, reps=1):
    sched = [s for _ in range(reps) for s in _schedule(nt)]
    return _build_core(RPC, sched, RD_MAP)


def _host_consts(W1, W2):
    def mmn(W):
        W = W.astype(np.float32)
        lo = W.min(1, keepdims=True)
        hi = W.max(1, keepdims=True)
        return (W - lo) / (hi - lo + np.float32(1e-8))

    eye12 = np.eye(12, dtype=np.float32)
    v1b = np.kron(eye12, (np.float32(1.0) - mmn(W1))).astype(np.float32)
    v2b = np.kron(eye12, (np.float32(1.0) - mmn(W2))).astype(np.float32)
    ident = np.eye(128, dtype=np.float32)
    return v1b, v2b, ident


def kernel(p1, p2, W1, W2, mask=None, **_unused):
    from concourse.bass_utils import run_bass_kernel_spmd

    if 'nc' not in _CACHED:
        _CACHED['nc'] = _build_nc()
    nc = _CACHED['nc']

    v1b, v2b, ident = _host_consts(W1, W2)
    p1 = np.ascontiguousarray(p1, dtype=np.float32)
    p2 = np.ascontiguousarray(p2, dtype=np.float32)

    in_maps = []
    for c in range(N_CORES):
        sl = slice(c * RPC, (c + 1) * RPC)
        in_maps.append({
            "p1": p1[sl], "p2": p2[sl],
            "v1b": v1b, "v2b": v2b, "ident": ident,
        })
    res = run_bass_kernel_spmd(nc, in_maps, list(range(N_CORES)))
    out = np.concatenate([res.results[c]["y"] for c in range(N_CORES)], axis=0)
    return out.astype(np.float32)


def _numpy_ref(p1, p2, W1, W2):
    def mmn(W):
        lo = W.min(1, keepdims=True); hi = W.max(1, keepdims=True)
        return (W - lo) / (hi - lo + np.float32(1e-8))
    A = p1 @ (1.0 - mmn(W1))
    Bv = p2 @ (1.0 - mmn(W2))
    st = np.maximum(A[:, :, None], Bv[:, None, :])
    Pk = np.ones((p1.shape[0], 19), np.float64)
    for i in range(10):
        for j in range(10):
            Pk[:, i + j] *= st[:, i, j]
    y = 1.0 - Pk
    return (y / (y.sum(1, keepdims=True) + 1e-9)).astype(np.float32)


def _simcheck():
    """CoreSim-exec validation on a small schedule."""
    from concourse.bass_interp import CoreSim
    rows = P * 16
    nc = _build_core(rows, [(0, 8), (P * 8, 8)], RD_MAP)
    rng = np.random.default_rng(1)
    p1 = rng.random((rows, 10), dtype=np.float32)
    p1 /= p1.sum(1, keepdims=True)
    p2 = rng.random((rows, 10), dtype=np.float32)
    p2 /= p2.sum(1, keepdims=True)
    W1 = rng.random((10, 10), dtype=np.float32)
    W2 = rng.random((10, 10), dtype=np.float32)
    v1b, v2b, ident = _host_consts(W1, W2)

    vals = {"p1": p1, "p2": p2, "v1b": v1b, "v2b": v2b, "ident": ident}
    bufs = {}
    for alloc in nc.m.functions[0].allocations:
        if hasattr(alloc, 'memorylocations') and alloc.memorylocations:
            for mem in alloc.memorylocations:
                if mem.name in vals:
                    a = np.ascontiguousarray(vals[mem.name])
                    bufs[mem.name] = a.view(np.uint8).reshape(tuple(mem.dims))
                elif mem.name == "y":
                    bufs["y"] = np.zeros(tuple(mem.dims), np.uint8)
    sim = CoreSim(nc, preallocated_bufs=bufs)
    sim.simulate()
    y = bufs["y"].view(np.float32).reshape(rows, 19)
    ref = _numpy_ref(p1, p2, W1, W2)
    rel = np.abs(y - ref) / np.maximum(np.abs(ref), 1e-12)
    print(f"simcheck: rel err max {rel.max():.3e}  sim time {sim.time:.0f} ns")
    assert rel.max() < 2e-2, "simcheck FAILED"
    print("simcheck PASSED")


if __name__ == "__main__":
    if "--simcheck" in sys.argv:
        _simcheck()
    else:
        rng = np.random.default_rng(0)
        p1 = rng.random((B, 10), dtype=np.float32)
        p1 /= p1.sum(1, keepdims=True)
        p2 = rng.random((B, 10), dtype=np.float32)
        p2 /= p2.sum(1, keepdims=True)
        W1 = rng.random((10, 10), dtype=np.float32)
        W2 = rng.random((10, 10), dtype=np.float32)
        y = kernel(p1, p2, W1, W2)
        print("kernel ran, y shape", y.shape, "sum", float(y.sum()))


# revision 27
# speedup vs baseline: 2.6225x; 1.0502x over previous
"""Trainium2 Bass kernel for nn_BaconAdditionReasoner (segment_reduce).

Math (per row b of 1M):
  a = p1 @ minmax(W1); b = p2 @ minmax(W2)           # [10] each
  s_ij = min(a_i, b_j); one_minus = 1 - clip(s)       # [10,10]
  y_k  = 1 - prod_{i+j=k} one_minus_ij                # 19 anti-diag bins
  y    = y / (sum_k y_k + 1e-9)

Kernel formulation (linear space, no Ln/Exp round-trip):
  A = p1 @ (1 - minmax(W1)); B = p2 @ (1 - minmax(W2))   # = one_minus rows
  one_minus_ij = max(A_i, B_j)   [clip never fires: A,B in (0.11, 0.84)]
  P_k = prod over anti-diagonal; y = (1 - P) / (19 + 1e-9 - sum P)

Engine split per tile (R rows/partition, rows on 128 partitions):
  PE    : 12-row-packed transposes + matmuls vs kron(I_12, V)
  ACT   : PSUM->SBUF copies (4-group-batched transpose spill and A|B
          copies), u = 1 - P
  DVE   : bf16 pair-packed max over mirror-bin pairs for bins 2..16
          (all strides +-1 -> 2x_1p double rate); fp32 edge bins
          0/1/17/18 (error amplification P/(1-P) is up to 4.1 there,
          <=0.54 for mid bins, so bf16 is only safe for the latter);
          bf16 multiply-trees for rows [0, Rd); reciprocal
  Pool  : multiply-trees for rows [Rd, R) via per-bin 2-dim APs (Pool
          ucode implements add/sub/mult/copy only -- no max/min/PSUM),
          the 19-bin add-tree for sum P, and y = u * r

Tiles are emitted software-pipelined (frontend of tile i+1 before the
backend of tile i, input DMAs two tiles ahead) so the in-order engine
queues interleave ready work instead of stalling on cross-engine waits.

Sharding: pure data parallel over 8 cores, 131072 rows each.
"""
import sys

if '/opt/trn_rl_repo' not in sys.path:
    sys.path.insert(0, '/opt/trn_rl_repo')

import numpy as np

B = 1048576
N_CORES = 8
RPC = B // N_CORES          # 131072 rows per core
P = 128                     # partitions
NT = 16                     # work units of 8192 rows (bench scaling)

# mirror-pair layout for bins (k, 18-k), k=1..8 (cnt c=k+1), plus bin 9.
# Bins 1/17 ride the bf16 path too: their amplification P/(1-P) is <= 0.98
# so the ~0.8% worst-case bf16 error stays well under the 2e-2 gate; only
# bins 0/18 (ratio up to 4.1) need fp32.
MID_KS = [1, 2, 3, 4, 5, 6, 7, 8]
PP_BASE = {}
_off = 0
for _k in MID_KS:
    PP_BASE[_k] = _off
    _off += 2 * (_k + 1)
PP_BASE[9] = _off
PP_W = _off + 10            # 94


def _groups_for(r):
    g = [12] * (r // 12)
    if r % 12:
        g.append(r % 12)
    return g


def _batches_for(r):
    """<=4 transpose groups (<=480 PSUM columns) per PSUM tile / copy."""
    gs = _groups_for(r)
    out, cur = [], []
    for g in gs:
        cur.append(g)
        if len(cur) == 4:
            out.append(cur)
            cur = []
    if cur:
        out.append(cur)
    return out


def _schedule(nt):
    rows = nt * P * 64
    rpp = rows // P
    # ramp tile sizes up at the start and down at the end: shortens the
    # pipeline fill (engines wait on tile 0's serial frontend) and the
    # drain (the last tile's serial backend tail)
    ramp_up, ramp_dn = [12, 12, 24, 48, 96], [96, 48, 36, 12]
    out, row0 = [], 0
    if rpp >= sum(ramp_up) + sum(ramp_dn) + 128:
        mid = rpp - sum(ramp_up) - sum(ramp_dn)
        plan = list(ramp_up) + [128] * (mid // 128)
        if mid % 128:
            plan.append(mid % 128)
        plan += ramp_dn
    else:
        plan = []
        left = rpp
        while left > 0:
            r = min(128, left)
            plan.append(r); left -= r
    for r in plan:
        out.append((row0, r)); row0 += P * r
    assert row0 == rows
    return out

_CACHED = {}


def _mkap(br, tile_ap, off, dims):
    return br.AP(tensor=tile_ap.tensor, offset=tile_ap.offset + off,
                 ap=[list(tile_ap.ap[0])] + [list(d) for d in dims])


def _emit_loads(nc, io, p1d, p2d, row0, R):
    import concourse.mybir as mybir
    F32 = mybir.dt.float32
    nrows = P * R
    p1v = p1d[row0:row0 + nrows, :].rearrange("(p r) c -> p (r c)", p=P)
    p2v = p2d[row0:row0 + nrows, :].rearrange("(p r) c -> p (r c)", p=P)
    p1t = io.tile([P, R * 10], F32, tag="p1t")
    p2t = io.tile([P, R * 10], F32, tag="p2t")
    nc.sync.dma_start(p1t[:], p1v)
    nc.sync.dma_start(p2t[:], p2v)
    return p1t, p2t


def _emit_front(nc, pools, consts, R, loaded):
    """Transposes, matmuls, PSUM->SBUF copies.

    Produces abb [P, R, 20] bf16 (A | B rows for the pair-packed max) and
    abe [P, R, 4] fp32 (A0 A9 B0 B9, for the fp32 edge bins 0/18)."""
    import bass_rust as _br
    import concourse.mybir as mybir

    F32 = mybir.dt.float32
    BF16 = mybir.dt.bfloat16
    io, ptp, abf, abb, pps, lptp, e4p, sm, tpp, mmp = pools
    v1t, v2t, idt = consts
    p1t, p2t = loaded

    abb_t = abb.tile([P, R, 20], BF16, tag="abb")
    abe = abf.tile([P, R, 4], F32, tag="abe")
    r0 = 0
    for batch in _batches_for(R):
        brows = sum(batch)
        bcols = brows * 10
        mma = mmp.tile([P, 480], F32, tag="mma")
        mmb = mmp.tile([P, 480], F32, tag="mmb")
        tpa = tpp.tile([120, 512], F32, tag="tpa")
        tpb = tpp.tile([120, 512], F32, tag="tpb")
        pta = ptp.tile([120, 512], F32, tag="pta")
        ptb = ptp.tile([120, 512], F32, tag="ptb")
        for src, vt, mm, tp, pt in ((p1t, v1t, mma, tpa, pta),
                                    (p2t, v2t, mmb, tpb, ptb)):
            off = 0
            full_k = 0
            for gi, gs in enumerate(batch):
                K = gs * 10
                nc.tensor.transpose(
                    tp[0:K, gi * 128:(gi + 1) * 128],
                    src[:, (r0 * 10 + off):(r0 * 10 + off + K)], idt[:])
                off += K
                if K == 120:
                    full_k += 1
            # batched PSUM->SBUF copy: full 120-row groups in one shot,
            # the ragged tail group (if any) separately
            if full_k:
                nc.scalar.copy(pt[:, 0:full_k * 128], tp[:, 0:full_k * 128])
            if full_k < len(batch):
                K = batch[-1] * 10
                nc.scalar.copy(pt[0:K, full_k * 128:full_k * 128 + 128],
                               tp[0:K, full_k * 128:full_k * 128 + 128])
            off = 0
            for gi, gs in enumerate(batch):
                K = gs * 10
                nc.tensor.matmul(mm[:, off:off + K],
                                 pt[0:K, gi * 128:(gi + 1) * 128],
                                 vt[0:K, 0:K], start=True, stop=True)
                off += K
        for mm, col in ((mma, 0), (mmb, 10)):
            dst = _mkap(_br, abb_t[:], r0 * 20 + col, [[20, brows], [1, 10]])
            nc.scalar.copy(
                dst, mm[:, 0:bcols].rearrange("p (r c) -> p r c", c=10))
        for mm, col in ((mma, 0), (mmb, 2)):
            dst = _mkap(_br, abe[:], r0 * 4 + col, [[4, brows], [1, 2]])
            src = _mkap(_br, mm[:, 0:bcols], 0, [[10, brows], [9, 2]])
            nc.scalar.copy(dst, src)
        r0 += brows
    return abb_t, abe


def _emit_back(nc, pools, front, yv, R, Rd):
    """Max, multiply-trees, normalize, store."""
    import bass_rust as _br
    import concourse.mybir as mybir
    from concourse.mybir import AluOpType

    F32 = mybir.dt.float32
    BF16 = mybir.dt.bfloat16
    io, ptp, abf, abb, pps, lptp, e4p, sm, tpp, mmp = pools
    abb_t, abe = front
    Rp = R - Rd

    def ap(tile_ap, off, dims):
        return _mkap(_br, tile_ap, off, dims)

    lpt = lptp.tile([P, R, 19], F32, tag="lpt")
    pp = pps.tile([P, R, PP_W], BF16, tag="pp")
    A, Bo = 0, 10
    for k in MID_KS:
        c = k + 1
        base = PP_BASE[k]
        nc.vector.tensor_tensor(
            ap(pp[:], base, [[PP_W, R], [c, 2], [1, c]]),
            ap(abb_t[:], A, [[20, R], [9 - k, 2], [1, c]]),
            ap(abb_t[:], Bo + k, [[20, R], [9 - k, 2], [-1, c]]),
            AluOpType.max)
    nc.vector.tensor_tensor(
        ap(pp[:], PP_BASE[9], [[PP_W, R], [1, 10]]),
        ap(abb_t[:], A, [[20, R], [1, 10]]),
        ap(abb_t[:], Bo + 9, [[20, R], [-1, 10]]),
        AluOpType.max)
    # edge bins 0/18 in fp32 (abe cols: A0 A9 B0 B9)
    nc.vector.tensor_tensor(
        ap(lpt[:], 0, [[19, R], [18, 2]]),
        ap(abe[:], 0, [[4, R], [1, 2]]),
        ap(abe[:], 2, [[4, R], [1, 2]]),
        AluOpType.max)
    # multiply-trees: rows [0, Rd) on DVE (bf16 2x pair APs), rows
    # [Rd, R) on Pool (per-bin 2-dim APs). Final level writes fp32 lpt.
    for k in MID_KS + [9]:
        c = k + 1 if k != 9 else 10
        base = PP_BASE[k]
        pdims = [[c, 2]] if k != 9 else []
        if Rd > 0:
            c0 = c
            while c0 > 2:
                fl, ce = c0 // 2, c0 - c0 // 2
                nc.vector.tensor_tensor(
                    ap(pp[:], base, [[PP_W, Rd]] + pdims + [[1, fl]]),
                    ap(pp[:], base, [[PP_W, Rd]] + pdims + [[1, fl]]),
                    ap(pp[:], base + ce, [[PP_W, Rd]] + pdims + [[1, fl]]),
                    AluOpType.mult)
                c0 = ce
            odims = [[18 - 2 * k, 2]] if k != 9 else []
            nc.vector.tensor_tensor(
                ap(lpt[:], k, [[19, Rd]] + odims + [[1, 1]]),
                ap(pp[:], base, [[PP_W, Rd]] + pdims + [[1, 1]]),
                ap(pp[:], base + 1, [[PP_W, Rd]] + pdims + [[1, 1]]),
                AluOpType.mult)
        if Rp > 0:
            for psel, kk in (((0, k), (1, 18 - k)) if k != 9 else ((0, 9),)):
                pb = Rd * PP_W + base + psel * c
                c0 = c
                while c0 > 2:
                    fl, ce = c0 // 2, c0 - c0 // 2
                    nc.gpsimd.tensor_tensor(
                        ap(pp[:], pb, [[PP_W, Rp], [1, fl]]),
                        ap(pp[:], pb, [[PP_W, Rp], [1, fl]]),
                        ap(pp[:], pb + ce, [[PP_W, Rp], [1, fl]]),
                        AluOpType.mult)
                    c0 = ce
                nc.gpsimd.tensor_tensor(
                    ap(lpt[:], Rd * 19 + kk, [[19, Rp], [1, 1]]),
                    ap(pp[:], pb, [[PP_W, Rp], [1, 1]]),
                    ap(pp[:], pb + 1, [[PP_W, Rp], [1, 1]]),
                    AluOpType.mult)

    # normalize: y = (1 - P) / (19 + 1e-9 - sum P); sum P as Pool add-tree
    st_ = sm.tile([P, R], F32, tag="S")
    sacc = sm.tile([P, R, 9], F32, tag="sacc")
    nc.gpsimd.tensor_tensor(sacc[:], lpt[:, :, 0:9], lpt[:, :, 10:19],
                            AluOpType.add)
    nc.gpsimd.tensor_tensor(sacc[:, :, 0:4], sacc[:, :, 0:4],
                            sacc[:, :, 5:9], AluOpType.add)
    nc.gpsimd.tensor_tensor(sacc[:, :, 0:2], sacc[:, :, 0:2],
                            sacc[:, :, 2:4], AluOpType.add)
    nc.gpsimd.tensor_tensor(sacc[:, :, 0:1], sacc[:, :, 0:1],
                            sacc[:, :, 1:2], AluOpType.add)
    nc.gpsimd.tensor_tensor(sacc[:, :, 4:5], sacc[:, :, 4:5],
                            lpt[:, :, 9:10], AluOpType.add)
    nc.gpsimd.tensor_tensor(st_[:].unsqueeze(2), sacc[:, :, 0:1],
                            sacc[:, :, 4:5], AluOpType.add)
    nc.gpsimd.tensor_scalar(st_[:], st_[:], -1.0, 19.0 + 1e-9,
                            AluOpType.mult, AluOpType.add)
    rt = sm.tile([P, R], F32, tag="r")
    nc.vector.reciprocal(rt[:], st_[:])
    nc.scalar.activation(
        lpt[:].rearrange("p r k -> p (r k)"),
        lpt[:].rearrange("p r k -> p (r k)"),
        mybir.ActivationFunctionType.Copy, bias=1.0, scale=-1.0)
    nc.gpsimd.tensor_tensor(
        lpt[:], lpt[:],
        rt[:].unsqueeze(2).broadcast_to((P, R, 19)),
        AluOpType.mult)
    nc.sync.dma_start(yv, lpt[:].rearrange("p r k -> p (r k)"))


def _build_core(rows_total, sched, rd_map):
    import concourse.mybir as mybir
    from concourse.bacc import Bacc
    from concourse.tile import TileContext

    F32 = mybir.dt.float32

    nc = Bacc()
    p1d = nc.dram_tensor("p1", [rows_total, 10], F32, kind="ExternalInput")
    p2d = nc.dram_tensor("p2", [rows_total, 10], F32, kind="ExternalInput")
    v1d = nc.dram_tensor("v1b", [120, 120], F32, kind="ExternalInput")
    v2d = nc.dram_tensor("v2b", [120, 120], F32, kind="ExternalInput")
    idd = nc.dram_tensor("ident", [128, 128], F32, kind="ExternalInput")
    yd = nc.dram_tensor("y", [rows_total, 19], F32, kind="ExternalOutput")

    with TileContext(nc) as tc:
        with (
            tc.tile_pool(name="const", bufs=1) as cpool,
            tc.tile_pool(name="io", bufs=4) as io,
            tc.tile_pool(name="pt", bufs=2) as ptp,
            tc.tile_pool(name="abf", bufs=3) as abf,
            tc.tile_pool(name="abb", bufs=3) as abb,
            tc.tile_pool(name="pps", bufs=2) as pps,
            tc.tile_pool(name="lpt", bufs=3) as lptp,
            tc.tile_pool(name="e4", bufs=2) as e4p,
            tc.tile_pool(name="sm", bufs=2) as sm,
            tc.tile_pool(name="tp", bufs=2, space="PSUM") as tpp,
            tc.tile_pool(name="mm", bufs=2, space="PSUM") as mmp,
        ):
            v1t = cpool.tile([120, 120], F32)
            v2t = cpool.tile([120, 120], F32)
            idt = cpool.tile([128, 128], F32)
            nc.sync.dma_start(idt[:], idd[:])
            nc.sync.dma_start(v1t[:], v1d[:])
            nc.sync.dma_start(v2t[:], v2d[:])
            pools = (io, ptp, abf, abb, pps, lptp, e4p, sm, tpp, mmp)
            consts = (v1t, v2t, idt)

            n = len(sched)
            loads, fronts = {}, {}
            for i in range(min(2, n)):
                loads[i] = _emit_loads(nc, io, p1d, p2d, *sched[i])
            if n:
                fronts[0] = _emit_front(nc, pools, consts, sched[0][1],
                                        loads.pop(0))
            for i in range(n):
                if i + 2 < n:
                    loads[i + 2] = _emit_loads(nc, io, p1d, p2d,
                                               *sched[i + 2])
                if i + 1 < n:
                    fronts[i + 1] = _emit_front(nc, pools, consts,
                                                sched[i + 1][1],
                                                loads.pop(i + 1))
                row0, R = sched[i]
                nrows = P * R
                yv = yd[row0:row0 + nrows, :].rearrange(
                    "(p r) k -> p (r k)", p=P)
                _emit_back(nc, pools, fronts.pop(i), yv, R, rd_map[R])

    nc.finalize()
    return nc


RD_MAP = {128: 28, 108: 24, 96: 21, 64: 14, 48: 10, 36: 8,
          32: 7, 24: 5, 16: 3, 12: 2, 8: 2}


def _build_nc(nt=NT, reps=1):
    sched = [s for _ in range(reps) for s in _schedule(nt)]
    return _build_core(RPC, sched, RD_MAP)


def _host_consts(W1, W2):
    def mmn(W):
        W = W.astype(np.float32)
        lo = W.min(1, keepdims=True)
        hi = W.max(1, keepdims=True)
        return (W - lo) / (hi - lo + np.float32(1e-8))

    eye12 = np.eye(12, dtype=np.float32)
    v1b = np.kron(eye12, (np.float32(1.0) - mmn(W1))).astype(np.float32)
    v2b = np.kron(eye12, (np.float32(1.0) - mmn(W2))).astype(np.float32)
    ident = np.eye(128, dtype=np.float32)
    return v1b, v2b, ident


def kernel(p1, p2, W1, W2, mask=None, **_unused):
    from concourse.bass_utils import run_bass_kernel_spmd

    if 'nc' not in _CACHED:
        _CACHED['nc'] = _build_nc()
    nc = _CACHED['nc']

    v1b, v2b, ident = _host_consts(W1, W2)
    p1 = np.ascontiguousarray(p1, dtype=np.float32)
    p2 = np.ascontiguousarray(p2, dtype=np.float32)

    in_maps = []
    for c in range(N_CORES):
        sl = slice(c * RPC, (c + 1) * RPC)
        in_maps.append({
            "p1": p1[sl], "p2": p2[sl],
            "v1b": v1b, "v2b": v2b, "ident": ident,
        })
    res = run_bass_kernel_spmd(nc, in_maps, list(range(N_CORES)))
    out = np.concatenate([res.results[c]["y"] for c in range(N_CORES)], axis=0)
    return out.astype(np.float32)


def _numpy_ref(p1, p2, W1, W2):
    def mmn(W):
        lo = W.min(1, keepdims=True); hi = W.max(1, keepdims=True)
        return (W - lo) / (hi - lo + np.float32(1e-8))
    A = p1 @ (1.0 - mmn(W1))
    Bv = p2 @ (1.0 - mmn(W2))
    st = np.maximum(A[:, :, None], Bv[:, None, :])
    Pk = np.ones((p1.shape[0], 19), np.float64)
    for i in range(10):
        for j in range(10):
            Pk[:, i + j] *= st[:, i, j]
    y = 1.0 - Pk
    return (y / (y.sum(1, keepdims=True) + 1e-9)).astype(np.float32)


def _simcheck():
    """CoreSim-exec validation on a small schedule."""
    from concourse.bass_interp import CoreSim
    rows = P * 16
    nc = _build_core(rows, [(0, 8), (P * 8, 8)], RD_MAP)
    rng = np.random.default_rng(1)
    p1 = rng.random((rows, 10), dtype=np.float32)
    p1 /= p1.sum(1, keepdims=True)
    p2 = rng.random((rows, 10), dtype=np.float32)
    p2 /= p2.sum(1, keepdims=True)
    W1 = rng.random((10, 10), dtype=np.float32)
    W2 = rng.random((10, 10), dtype=np.float32)
    v1b, v2b, ident = _host_consts(W1, W2)

    vals = {"p1": p1, "p2": p2, "v1b": v1b, "v2b": v2b, "ident": ident}
    bufs = {}
    for alloc in nc.m.functions[0].allocations:
        if hasattr(alloc, 'memorylocations') and alloc.memorylocations:
            for mem in alloc.memorylocations:
                if mem.name in vals:
                    a = np.ascontiguousarray(vals[mem.name])
                    bufs[mem.name] = a.view(np.uint8).reshape(tuple(mem.dims))
                elif mem.name == "y":
                    bufs["y"] = np.zeros(tuple(mem.dims), np.uint8)
    sim = CoreSim(nc, preallocated_bufs=bufs)
    sim.simulate()
    y = bufs["y"].view(np.float32).reshape(rows, 19)
    ref = _numpy_ref(p1, p2, W1, W2)
    rel = np.abs(y - ref) / np.maximum(np.abs(ref), 1e-12)
    print(f"simcheck: rel err max {rel.max():.3e}  sim time {sim.time:.0f} ns")
    assert rel.max() < 2e-2, "simcheck FAILED"
    print("simcheck PASSED")


if __name__ == "__main__":
    if "--simcheck" in sys.argv:
        _simcheck()
    else:
        rng = np.random.default_rng(0)
        p1 = rng.random((B, 10), dtype=np.float32)
        p1 /= p1.sum(1, keepdims=True)
        p2 = rng.random((B, 10), dtype=np.float32)
        p2 /= p2.sum(1, keepdims=True)
        W1 = rng.random((10, 10), dtype=np.float32)
        W2 = rng.random((10, 10), dtype=np.float32)
        y = kernel(p1, p2, W1, W2)
        print("kernel ran, y shape", y.shape, "sum", float(y.sum()))


# revision 31
# speedup vs baseline: 2.6384x; 1.0061x over previous
"""Trainium2 Bass kernel for nn_BaconAdditionReasoner (segment_reduce).

Math (per row b of 1M):
  a = p1 @ minmax(W1); b = p2 @ minmax(W2)           # [10] each
  s_ij = min(a_i, b_j); one_minus = 1 - clip(s)       # [10,10]
  y_k  = 1 - prod_{i+j=k} one_minus_ij                # 19 anti-diag bins
  y    = y / (sum_k y_k + 1e-9)

Kernel formulation (linear space, no Ln/Exp round-trip):
  A = p1 @ (1 - minmax(W1)); B = p2 @ (1 - minmax(W2))   # = one_minus rows
  one_minus_ij = max(A_i, B_j)   [clip never fires: A,B in (0.11, 0.84)]
  P_k = prod over anti-diagonal; y = (1 - P) / (19 + 1e-9 - sum P)

Engine split per tile (R rows/partition, rows on 128 partitions):
  PE    : 12-row-packed transposes + matmuls vs kron(I_12, V)
  ACT   : PSUM->SBUF copies (4-group-batched transpose spill and A|B
          copies), u = 1 - P
  DVE   : bf16 pair-packed max over mirror-bin pairs for bins 2..16
          (all strides +-1 -> 2x_1p double rate); fp32 edge bins
          0/1/17/18 (error amplification P/(1-P) is up to 4.1 there,
          <=0.54 for mid bins, so bf16 is only safe for the latter);
          bf16 multiply-trees for rows [0, Rd); reciprocal
  Pool  : multiply-trees for rows [Rd, R) via per-bin 2-dim APs (Pool
          ucode implements add/sub/mult/copy only -- no max/min/PSUM),
          the 19-bin add-tree for sum P, and y = u * r

Tiles are emitted software-pipelined (frontend of tile i+1 before the
backend of tile i, input DMAs two tiles ahead) so the in-order engine
queues interleave ready work instead of stalling on cross-engine waits.

Sharding: pure data parallel over 8 cores, 131072 rows each.
"""
import sys

if '/opt/trn_rl_repo' not in sys.path:
    sys.path.insert(0, '/opt/trn_rl_repo')

import numpy as np

B = 1048576
N_CORES = 8
RPC = B // N_CORES          # 131072 rows per core
P = 128                     # partitions
NT = 16                     # work units of 8192 rows (bench scaling)

# Uniform-slot layout for bins (k, 18-k), k=1..8 (cnt c=k+1), plus bin 9:
# bin k lives at slot (k-1)*10, its mirror at +90, bin 9 at 80. The
# constant slot/mirror strides let multiply-tree levels merge across
# adjacent pairs (equal fl; the ce offsets line up as stride-11 APs).
# Bins 1/17 ride the bf16 path: their amplification P/(1-P) is <= 0.98 so
# the ~0.8% worst-case bf16 error stays well under the 2e-2 gate; only
# bins 0/18 (ratio up to 4.1) need fp32.
MID_KS = [1, 2, 3, 4, 5, 6, 7, 8]
PP_SLOT = {k: (k - 1) * 10 for k in MID_KS}
PP_SLOT[9] = 80
PP_MIR = 90
PP_W = 180


def _groups_for(r):
    g = [12] * (r // 12)
    if r % 12:
        g.append(r % 12)
    return g


def _batches_for(r):
    """<=4 transpose groups (<=480 PSUM columns) per PSUM tile / copy."""
    gs = _groups_for(r)
    out, cur = [], []
    for g in gs:
        cur.append(g)
        if len(cur) == 4:
            out.append(cur)
            cur = []
    if cur:
        out.append(cur)
    return out


def _schedule(nt):
    rows = nt * P * 64
    rpp = rows // P
    # ramp tile sizes up at the start and down at the end: shortens the
    # pipeline fill (engines wait on tile 0's serial frontend) and the
    # drain (the last tile's serial backend tail)
    ramp_up, ramp_dn = [12, 12, 24, 48, 96], [96, 48, 36, 12]
    out, row0 = [], 0
    if rpp >= sum(ramp_up) + sum(ramp_dn) + 128:
        mid = rpp - sum(ramp_up) - sum(ramp_dn)
        plan = list(ramp_up) + [128] * (mid // 128)
        if mid % 128:
            plan.append(mid % 128)
        plan += ramp_dn
    else:
        plan = []
        left = rpp
        while left > 0:
            r = min(128, left)
            plan.append(r); left -= r
    for r in plan:
        out.append((row0, r)); row0 += P * r
    assert row0 == rows
    return out

_CACHED = {}


def _mkap(br, tile_ap, off, dims):
    return br.AP(tensor=tile_ap.tensor, offset=tile_ap.offset + off,
                 ap=[list(tile_ap.ap[0])] + [list(d) for d in dims])


def _emit_loads(nc, io, p1d, p2d, row0, R):
    import concourse.mybir as mybir
    F32 = mybir.dt.float32
    nrows = P * R
    p1v = p1d[row0:row0 + nrows, :].rearrange("(p r) c -> p (r c)", p=P)
    p2v = p2d[row0:row0 + nrows, :].rearrange("(p r) c -> p (r c)", p=P)
    p1t = io.tile([P, R * 10], F32, tag="p1t")
    p2t = io.tile([P, R * 10], F32, tag="p2t")
    nc.sync.dma_start(p1t[:], p1v)
    nc.sync.dma_start(p2t[:], p2v)
    return p1t, p2t


def _emit_front(nc, pools, consts, R, loaded):
    """Transposes, matmuls, PSUM->SBUF copies.

    Produces abb [P, R, 20] bf16 (A | B rows for the pair-packed max) and
    abe [P, R, 4] fp32 (A0 A9 B0 B9, for the fp32 edge bins 0/18)."""
    import bass_rust as _br
    import concourse.mybir as mybir

    F32 = mybir.dt.float32
    BF16 = mybir.dt.bfloat16
    io, ptp, abf, abb, pps, lptp, e4p, sm, tpp, mmp = pools
    v1t, v2t, idt = consts
    p1t, p2t = loaded

    abb_t = abb.tile([P, R, 20], BF16, tag="abb")
    abe = abf.tile([P, R, 4], F32, tag="abe")
    r0 = 0
    for batch in _batches_for(R):
        brows = sum(batch)
        bcols = brows * 10
        mma = mmp.tile([P, 480], F32, tag="mma")
        mmb = mmp.tile([P, 480], F32, tag="mmb")
        tpa = tpp.tile([120, 512], F32, tag="tpa")
        tpb = tpp.tile([120, 512], F32, tag="tpb")
        pta = ptp.tile([120, 512], F32, tag="pta")
        ptb = ptp.tile([120, 512], F32, tag="ptb")
        for src, vt, mm, tp, pt in ((p1t, v1t, mma, tpa, pta),
                                    (p2t, v2t, mmb, tpb, ptb)):
            off = 0
            full_k = 0
            for gi, gs in enumerate(batch):
                K = gs * 10
                nc.tensor.transpose(
                    tp[0:K, gi * 128:(gi + 1) * 128],
                    src[:, (r0 * 10 + off):(r0 * 10 + off + K)], idt[:])
                off += K
                if K == 120:
                    full_k += 1
            # batched PSUM->SBUF copy: full 120-row groups in one shot,
            # the ragged tail group (if any) separately
            if full_k:
                nc.scalar.copy(pt[:, 0:full_k * 128], tp[:, 0:full_k * 128])
            if full_k < len(batch):
                K = batch[-1] * 10
                nc.scalar.copy(pt[0:K, full_k * 128:full_k * 128 + 128],
                               tp[0:K, full_k * 128:full_k * 128 + 128])
            off = 0
            for gi, gs in enumerate(batch):
                K = gs * 10
                nc.tensor.matmul(mm[:, off:off + K],
                                 pt[0:K, gi * 128:(gi + 1) * 128],
                                 vt[0:K, 0:K], start=True, stop=True)
                off += K
        for mm, col in ((mma, 0), (mmb, 10)):
            dst = _mkap(_br, abb_t[:], r0 * 20 + col, [[20, brows], [1, 10]])
            nc.scalar.copy(
                dst, mm[:, 0:bcols].rearrange("p (r c) -> p r c", c=10))
        for mm, col in ((mma, 0), (mmb, 2)):
            dst = _mkap(_br, abe[:], r0 * 4 + col, [[4, brows], [1, 2]])
            src = _mkap(_br, mm[:, 0:bcols], 0, [[10, brows], [9, 2]])
            nc.scalar.copy(dst, src)
        r0 += brows
    return abb_t, abe


def _emit_back(nc, pools, front, yv, R, Rd):
    """Max, multiply-trees, normalize, store."""
    import bass_rust as _br
    import concourse.mybir as mybir
    from concourse.mybir import AluOpType

    F32 = mybir.dt.float32
    BF16 = mybir.dt.bfloat16
    io, ptp, abf, abb, pps, lptp, e4p, sm, tpp, mmp = pools
    abb_t, abe = front
    Rp = R - Rd

    def ap(tile_ap, off, dims):
        return _mkap(_br, tile_ap, off, dims)

    lpt = lptp.tile([P, R, 19], F32, tag="lpt")
    pp = pps.tile([P, R, PP_W], BF16, tag="pp")
    A, Bo = 0, 10
    for k in MID_KS:
        c = k + 1
        nc.vector.tensor_tensor(
            ap(pp[:], PP_SLOT[k], [[PP_W, R], [PP_MIR, 2], [1, c]]),
            ap(abb_t[:], A, [[20, R], [9 - k, 2], [1, c]]),
            ap(abb_t[:], Bo + k, [[20, R], [9 - k, 2], [-1, c]]),
            AluOpType.max)
    nc.vector.tensor_tensor(
        ap(pp[:], PP_SLOT[9], [[PP_W, R], [1, 10]]),
        ap(abb_t[:], A, [[20, R], [1, 10]]),
        ap(abb_t[:], Bo + 9, [[20, R], [-1, 10]]),
        AluOpType.max)
    # edge bins 0/18 in fp32 (abe cols: A0 A9 B0 B9)
    nc.vector.tensor_tensor(
        ap(lpt[:], 0, [[19, R], [18, 2]]),
        ap(abe[:], 0, [[4, R], [1, 2]]),
        ap(abe[:], 2, [[4, R], [1, 2]]),
        AluOpType.max)
    # multiply-trees. DVE rows [0, Rd): levels merged across adjacent
    # pairs (uniform slot layout; in1 offsets align at stride 11 where ce
    # increments). Pool rows [Rd, R): per-bin 2-dim APs (Pool ucode has
    # mult but not max/min). Final level writes fp32 into lpt.
    if Rd > 0:
        M2 = [[PP_MIR, 2]]
        mul = AluOpType.mult
        def dv(o_off, o_dims, i_off, i_dims):
            nc.vector.tensor_tensor(ap(pp[:], o_off, [[PP_W, Rd]] + o_dims),
                                    ap(pp[:], o_off, [[PP_W, Rd]] + o_dims),
                                    ap(pp[:], i_off, [[PP_W, Rd]] + i_dims),
                                    mul)
        # L1
        dv(10, M2 + [[1, 1]], 12, M2 + [[1, 1]])                      # c3
        dv(20, M2 + [[10, 2], [1, 2]], 22, M2 + [[11, 2], [1, 2]])    # c4,c5
        dv(40, M2 + [[10, 2], [1, 3]], 43, M2 + [[11, 2], [1, 3]])    # c6,c7
        dv(60, M2 + [[10, 2], [1, 4]], 64, M2 + [[11, 2], [1, 4]])    # c8,c9
        dv(80, [[1, 5]], 85, [[1, 5]])                                # c10
        # L2
        dv(30, M2 + [[10, 2], [1, 1]], 32, M2 + [[10, 2], [1, 1]])    # c5,c6
        dv(50, M2 + [[10, 2], [1, 2]], 52, M2 + [[10, 2], [1, 2]])    # c7,c8
        dv(70, M2 + [[1, 2]], 73, M2 + [[1, 2]])                      # c9
        dv(80, [[1, 2]], 83, [[1, 2]])                                # c10
        # L3
        dv(70, M2 + [[1, 1]], 72, M2 + [[1, 1]])                      # c9
        dv(80, [[1, 1]], 82, [[1, 1]])                                # c10
        # finals -> lpt (fp32)
        for k in MID_KS:
            nc.vector.tensor_tensor(
                ap(lpt[:], k, [[19, Rd], [18 - 2 * k, 2], [1, 1]]),
                ap(pp[:], PP_SLOT[k], [[PP_W, Rd]] + M2 + [[1, 1]]),
                ap(pp[:], PP_SLOT[k] + 1, [[PP_W, Rd]] + M2 + [[1, 1]]),
                mul)
        nc.vector.tensor_tensor(
            ap(lpt[:], 9, [[19, Rd], [1, 1]]),
            ap(pp[:], 80, [[PP_W, Rd], [1, 1]]),
            ap(pp[:], 81, [[PP_W, Rd], [1, 1]]), mul)
    if Rp > 0:
        for k in MID_KS + [9]:
            c = k + 1 if k != 9 else 10
            sides = (((0, k), (1, 18 - k)) if k != 9 else ((0, 9),))
            for psel, kk in sides:
                pb = Rd * PP_W + PP_SLOT[k] + psel * PP_MIR
                c0 = c
                while c0 > 2:
                    fl, ce = c0 // 2, c0 - c0 // 2
                    nc.gpsimd.tensor_tensor(
                        ap(pp[:], pb, [[PP_W, Rp], [1, fl]]),
                        ap(pp[:], pb, [[PP_W, Rp], [1, fl]]),
                        ap(pp[:], pb + ce, [[PP_W, Rp], [1, fl]]),
                        AluOpType.mult)
                    c0 = ce
                nc.gpsimd.tensor_tensor(
                    ap(lpt[:], Rd * 19 + kk, [[19, Rp], [1, 1]]),
                    ap(pp[:], pb, [[PP_W, Rp], [1, 1]]),
                    ap(pp[:], pb + 1, [[PP_W, Rp], [1, 1]]),
                    AluOpType.mult)

    # normalize: y = (1 - P) / (19 + 1e-9 - sum P); sum P as Pool add-tree
    st_ = sm.tile([P, R], F32, tag="S")
    sacc = sm.tile([P, R, 9], F32, tag="sacc")
    nc.gpsimd.tensor_tensor(sacc[:], lpt[:, :, 0:9], lpt[:, :, 10:19],
                            AluOpType.add)
    nc.gpsimd.tensor_tensor(sacc[:, :, 0:4], sacc[:, :, 0:4],
                            sacc[:, :, 5:9], AluOpType.add)
    nc.gpsimd.tensor_tensor(sacc[:, :, 0:2], sacc[:, :, 0:2],
                            sacc[:, :, 2:4], AluOpType.add)
    nc.gpsimd.tensor_tensor(sacc[:, :, 0:1], sacc[:, :, 0:1],
                            sacc[:, :, 1:2], AluOpType.add)
    nc.gpsimd.tensor_tensor(sacc[:, :, 4:5], sacc[:, :, 4:5],
                            lpt[:, :, 9:10], AluOpType.add)
    nc.gpsimd.tensor_tensor(st_[:].unsqueeze(2), sacc[:, :, 0:1],
                            sacc[:, :, 4:5], AluOpType.add)
    nc.gpsimd.tensor_scalar(st_[:], st_[:], -1.0, 19.0 + 1e-9,
                            AluOpType.mult, AluOpType.add)
    rt = sm.tile([P, R], F32, tag="r")
    nc.vector.reciprocal(rt[:], st_[:])
    nc.scalar.activation(
        lpt[:].rearrange("p r k -> p (r k)"),
        lpt[:].rearrange("p r k -> p (r k)"),
        mybir.ActivationFunctionType.Copy, bias=1.0, scale=-1.0)
    nc.gpsimd.tensor_tensor(
        lpt[:], lpt[:],
        rt[:].unsqueeze(2).broadcast_to((P, R, 19)),
        AluOpType.mult)
    nc.sync.dma_start(yv, lpt[:].rearrange("p r k -> p (r k)"))


def _build_core(rows_total, sched, rd_map):
    import concourse.mybir as mybir
    from concourse.bacc import Bacc
    from concourse.tile import TileContext

    F32 = mybir.dt.float32

    nc = Bacc()
    p1d = nc.dram_tensor("p1", [rows_total, 10], F32, kind="ExternalInput")
    p2d = nc.dram_tensor("p2", [rows_total, 10], F32, kind="ExternalInput")
    v1d = nc.dram_tensor("v1b", [120, 120], F32, kind="ExternalInput")
    v2d = nc.dram_tensor("v2b", [120, 120], F32, kind="ExternalInput")
    idd = nc.dram_tensor("ident", [128, 128], F32, kind="ExternalInput")
    yd = nc.dram_tensor("y", [rows_total, 19], F32, kind="ExternalOutput")

    with TileContext(nc) as tc:
        with (
            tc.tile_pool(name="const", bufs=1) as cpool,
            tc.tile_pool(name="io", bufs=3) as io,
            tc.tile_pool(name="pt", bufs=2) as ptp,
            tc.tile_pool(name="abf", bufs=3) as abf,
            tc.tile_pool(name="abb", bufs=3) as abb,
            tc.tile_pool(name="pps", bufs=2) as pps,
            tc.tile_pool(name="lpt", bufs=3) as lptp,
            tc.tile_pool(name="e4", bufs=2) as e4p,
            tc.tile_pool(name="sm", bufs=2) as sm,
            tc.tile_pool(name="tp", bufs=2, space="PSUM") as tpp,
            tc.tile_pool(name="mm", bufs=2, space="PSUM") as mmp,
        ):
            v1t = cpool.tile([120, 120], F32)
            v2t = cpool.tile([120, 120], F32)
            idt = cpool.tile([128, 128], F32)
            nc.sync.dma_start(idt[:], idd[:])
            nc.sync.dma_start(v1t[:], v1d[:])
            nc.sync.dma_start(v2t[:], v2d[:])
            pools = (io, ptp, abf, abb, pps, lptp, e4p, sm, tpp, mmp)
            consts = (v1t, v2t, idt)

            n = len(sched)
            loads, fronts = {}, {}
            for i in range(min(2, n)):
                loads[i] = _emit_loads(nc, io, p1d, p2d, *sched[i])
            if n:
                fronts[0] = _emit_front(nc, pools, consts, sched[0][1],
                                        loads.pop(0))
            for i in range(n):
                if i + 2 < n:
                    loads[i + 2] = _emit_loads(nc, io, p1d, p2d,
                                               *sched[i + 2])
                if i + 1 < n:
                    fronts[i + 1] = _emit_front(nc, pools, consts,
                                                sched[i + 1][1],
                                                loads.pop(i + 1))
                row0, R = sched[i]
                nrows = P * R
                yv = yd[row0:row0 + nrows, :].rearrange(
                    "(p r) k -> p (r k)", p=P)
                _emit_back(nc, pools, fronts.pop(i), yv, R, rd_map[R])

    nc.finalize()
    return nc


RD_MAP = {128: 28, 108: 24, 96: 21, 64: 14, 48: 10, 36: 8,
          32: 7, 24: 5, 16: 3, 12: 2, 8: 2}


def _build_nc(nt=NT, reps=1):
    sched = [s for _ in range(reps) for s in _schedule(nt)]
    return _build_core(RPC, sched, RD_MAP)


def _host_consts(W1, W2):
    def mmn(W):
        W = W.astype(np.float32)
        lo = W.min(1, keepdims=True)
        hi = W.max(1, keepdims=True)
        return (W - lo) / (hi - lo + np.float32(1e-8))

    eye12 = np.eye(12, dtype=np.float32)
    v1b = np.kron(eye12, (np.float32(1.0) - mmn(W1))).astype(np.float32)
    v2b = np.kron(eye12, (np.float32(1.0) - mmn(W2))).astype(np.float32)
    ident = np.eye(128, dtype=np.float32)
    return v1b, v2b, ident


def kernel(p1, p2, W1, W2, mask=None, **_unused):
    from concourse.bass_utils import run_bass_kernel_spmd

    if 'nc' not in _CACHED:
        _CACHED['nc'] = _build_nc()
    nc = _CACHED['nc']

    v1b, v2b, ident = _host_consts(W1, W2)
    p1 = np.ascontiguousarray(p1, dtype=np.float32)
    p2 = np.ascontiguousarray(p2, dtype=np.float32)

    in_maps = []
    for c in range(N_CORES):
        sl = slice(c * RPC, (c + 1) * RPC)
        in_maps.append({
            "p1": p1[sl], "p2": p2[sl],
            "v1b": v1b, "v2b": v2b, "ident": ident,
        })
    res = run_bass_kernel_spmd(nc, in_maps, list(range(N_CORES)))
    out = np.concatenate([res.results[c]["y"] for c in range(N_CORES)], axis=0)
    return out.astype(np.float32)


def _numpy_ref(p1, p2, W1, W2):
    def mmn(W):
        lo = W.min(1, keepdims=True); hi = W.max(1, keepdims=True)
        return (W - lo) / (hi - lo + np.float32(1e-8))
    A = p1 @ (1.0 - mmn(W1))
    Bv = p2 @ (1.0 - mmn(W2))
    st = np.maximum(A[:, :, None], Bv[:, None, :])
    Pk = np.ones((p1.shape[0], 19), np.float64)
    for i in range(10):
        for j in range(10):
            Pk[:, i + j] *= st[:, i, j]
    y = 1.0 - Pk
    return (y / (y.sum(1, keepdims=True) + 1e-9)).astype(np.float32)


def _simcheck():
    """CoreSim-exec validation on a small schedule."""
    from concourse.bass_interp import CoreSim
    rows = P * 16
    nc = _build_core(rows, [(0, 8), (P * 8, 8)], RD_MAP)
    rng = np.random.default_rng(1)
    p1 = rng.random((rows, 10), dtype=np.float32)
    p1 /= p1.sum(1, keepdims=True)
    p2 = rng.random((rows, 10), dtype=np.float32)
    p2 /= p2.sum(1, keepdims=True)
    W1 = rng.random((10, 10), dtype=np.float32)
    W2 = rng.random((10, 10), dtype=np.float32)
    v1b, v2b, ident = _host_consts(W1, W2)

    vals = {"p1": p1, "p2": p2, "v1b": v1b, "v2b": v2b, "ident": ident}
    bufs = {}
    for alloc in nc.m.functions[0].allocations:
        if hasattr(alloc, 'memorylocations') and alloc.memorylocations:
            for mem in alloc.memorylocations:
                if mem.name in vals:
                    a = np.ascontiguousarray(vals[mem.name])
                    bufs[mem.name] = a.view(np.uint8).reshape(tuple(mem.dims))
                elif mem.name == "y":
                    bufs["y"] = np.zeros(tuple(mem.dims), np.uint8)
    sim = CoreSim(nc, preallocated_bufs=bufs)
    sim.simulate()
    y = bufs["y"].view(np.float32).reshape(rows, 19)
    ref = _numpy_ref(p1, p2, W1, W2)
    rel = np.abs(y - ref) / np.maximum(np.abs(ref), 1e-12)
    print(f"simcheck: rel err max {rel.max():.3e}  sim time {sim.time:.0f} ns")
    assert rel.max() < 2e-2, "simcheck FAILED"
    print("simcheck PASSED")


if __name__ == "__main__":
    if "--simcheck" in sys.argv:
        _simcheck()
    else:
        rng = np.random.default_rng(0)
        p1 = rng.random((B, 10), dtype=np.float32)
        p1 /= p1.sum(1, keepdims=True)
        p2 = rng.random((B, 10), dtype=np.float32)
        p2 /= p2.sum(1, keepdims=True)
        W1 = rng.random((10, 10), dtype=np.float32)
        W2 = rng.random((10, 10), dtype=np.float32)
        y = kernel(p1, p2, W1, W2)
        print("kernel ran, y shape", y.shape, "sum", float(y.sum()))


# revision 32
# speedup vs baseline: 2.6449x; 1.0025x over previous
"""Trainium2 Bass kernel for nn_BaconAdditionReasoner (segment_reduce).

Math (per row b of 1M):
  a = p1 @ minmax(W1); b = p2 @ minmax(W2)           # [10] each
  s_ij = min(a_i, b_j); one_minus = 1 - clip(s)       # [10,10]
  y_k  = 1 - prod_{i+j=k} one_minus_ij                # 19 anti-diag bins
  y    = y / (sum_k y_k + 1e-9)

Kernel formulation (linear space, no Ln/Exp round-trip):
  A = p1 @ (1 - minmax(W1)); B = p2 @ (1 - minmax(W2))   # = one_minus rows
  one_minus_ij = max(A_i, B_j)   [clip never fires: A,B in (0.11, 0.84)]
  P_k = prod over anti-diagonal; y = (1 - P) / (19 + 1e-9 - sum P)

Engine split per tile (R rows/partition, rows on 128 partitions):
  PE    : 12-row-packed transposes + matmuls vs kron(I_12, V)
  ACT   : PSUM->SBUF copies (4-group-batched transpose spill and A|B
          copies), u = 1 - P
  DVE   : bf16 pair-packed max over mirror-bin pairs for bins 2..16
          (all strides +-1 -> 2x_1p double rate); fp32 edge bins
          0/1/17/18 (error amplification P/(1-P) is up to 4.1 there,
          <=0.54 for mid bins, so bf16 is only safe for the latter);
          bf16 multiply-trees for rows [0, Rd); reciprocal
  Pool  : multiply-trees for rows [Rd, R) via per-bin 2-dim APs (Pool
          ucode implements add/sub/mult/copy only -- no max/min/PSUM),
          the 19-bin add-tree for sum P, and y = u * r

Tiles are emitted software-pipelined (frontend of tile i+1 before the
backend of tile i, input DMAs two tiles ahead) so the in-order engine
queues interleave ready work instead of stalling on cross-engine waits.

Sharding: pure data parallel over 8 cores, 131072 rows each.
"""
import sys

if '/opt/trn_rl_repo' not in sys.path:
    sys.path.insert(0, '/opt/trn_rl_repo')

import numpy as np

B = 1048576
N_CORES = 8
RPC = B // N_CORES          # 131072 rows per core
P = 128                     # partitions
NT = 16                     # work units of 8192 rows (bench scaling)

# Uniform-slot layout for bins (k, 18-k), k=1..8 (cnt c=k+1), plus bin 9:
# bin k lives at slot (k-1)*10, its mirror at +90, bin 9 at 80. The
# constant slot/mirror strides let multiply-tree levels merge across
# adjacent pairs (equal fl; the ce offsets line up as stride-11 APs).
# Bins 1/17 ride the bf16 path: their amplification P/(1-P) is <= 0.98 so
# the ~0.8% worst-case bf16 error stays well under the 2e-2 gate; only
# bins 0/18 (ratio up to 4.1) need fp32.
MID_KS = [1, 2, 3, 4, 5, 6, 7, 8]
PP_SLOT = {k: (k - 1) * 10 for k in MID_KS}
PP_SLOT[9] = 80
PP_MIR = 90
PP_W = 180


def _groups_for(r):
    g = [12] * (r // 12)
    if r % 12:
        g.append(r % 12)
    return g


def _batches_for(r):
    """<=4 transpose groups (<=480 PSUM columns) per PSUM tile / copy."""
    gs = _groups_for(r)
    out, cur = [], []
    for g in gs:
        cur.append(g)
        if len(cur) == 4:
            out.append(cur)
            cur = []
    if cur:
        out.append(cur)
    return out


def _schedule(nt):
    rows = nt * P * 64
    rpp = rows // P
    # ramp tile sizes up at the start and down at the end: shortens the
    # pipeline fill (engines wait on tile 0's serial frontend) and the
    # drain (the last tile's serial backend tail)
    ramp_up, ramp_dn = [12, 12, 24, 48, 96], [96, 48, 36, 12]
    out, row0 = [], 0
    if rpp >= sum(ramp_up) + sum(ramp_dn) + 128:
        mid = rpp - sum(ramp_up) - sum(ramp_dn)
        plan = list(ramp_up) + [128] * (mid // 128)
        if mid % 128:
            plan.append(mid % 128)
        plan += ramp_dn
    else:
        plan = []
        left = rpp
        while left > 0:
            r = min(128, left)
            plan.append(r); left -= r
    for r in plan:
        out.append((row0, r)); row0 += P * r
    assert row0 == rows
    return out

_CACHED = {}


def _mkap(br, tile_ap, off, dims):
    return br.AP(tensor=tile_ap.tensor, offset=tile_ap.offset + off,
                 ap=[list(tile_ap.ap[0])] + [list(d) for d in dims])


def _emit_loads(nc, io, p1d, p2d, row0, R):
    import concourse.mybir as mybir
    F32 = mybir.dt.float32
    nrows = P * R
    p1v = p1d[row0:row0 + nrows, :].rearrange("(p r) c -> p (r c)", p=P)
    p2v = p2d[row0:row0 + nrows, :].rearrange("(p r) c -> p (r c)", p=P)
    p1t = io.tile([P, R * 10], F32, tag="p1t")
    p2t = io.tile([P, R * 10], F32, tag="p2t")
    nc.sync.dma_start(p1t[:], p1v)
    nc.sync.dma_start(p2t[:], p2v)
    return p1t, p2t


def _emit_front(nc, pools, consts, R, loaded):
    """Transposes, matmuls, PSUM->SBUF copies.

    Produces abb [P, R, 20] bf16 (A | B rows for the pair-packed max) and
    abe [P, R, 4] fp32 (A0 A9 B0 B9, for the fp32 edge bins 0/18)."""
    import bass_rust as _br
    import concourse.mybir as mybir

    F32 = mybir.dt.float32
    BF16 = mybir.dt.bfloat16
    io, ptp, abf, abb, pps, lptp, e4p, sm, tpp, mmp = pools
    v1t, v2t, idt = consts
    p1t, p2t = loaded

    abb_t = abb.tile([P, R, 20], BF16, tag="abb")
    abe = abf.tile([P, R, 4], F32, tag="abe")
    r0 = 0
    for batch in _batches_for(R):
        brows = sum(batch)
        bcols = brows * 10
        mma = mmp.tile([P, 480], F32, tag="mma")
        mmb = mmp.tile([P, 480], F32, tag="mmb")
        tpa = tpp.tile([120, 512], F32, tag="tpa")
        tpb = tpp.tile([120, 512], F32, tag="tpb")
        pta = ptp.tile([120, 512], F32, tag="pta")
        ptb = ptp.tile([120, 512], F32, tag="ptb")
        for src, vt, mm, tp, pt in ((p1t, v1t, mma, tpa, pta),
                                    (p2t, v2t, mmb, tpb, ptb)):
            off = 0
            full_k = 0
            for gi, gs in enumerate(batch):
                K = gs * 10
                nc.tensor.transpose(
                    tp[0:K, gi * 128:(gi + 1) * 128],
                    src[:, (r0 * 10 + off):(r0 * 10 + off + K)], idt[:])
                off += K
                if K == 120:
                    full_k += 1
            # batched PSUM->SBUF copy: full 120-row groups in one shot,
            # the ragged tail group (if any) separately
            if full_k:
                nc.scalar.copy(pt[:, 0:full_k * 128], tp[:, 0:full_k * 128])
            if full_k < len(batch):
                K = batch[-1] * 10
                nc.scalar.copy(pt[0:K, full_k * 128:full_k * 128 + 128],
                               tp[0:K, full_k * 128:full_k * 128 + 128])
            off = 0
            for gi, gs in enumerate(batch):
                K = gs * 10
                nc.tensor.matmul(mm[:, off:off + K],
                                 pt[0:K, gi * 128:(gi + 1) * 128],
                                 vt[0:K, 0:K], start=True, stop=True)
                off += K
        for mm, col in ((mma, 0), (mmb, 10)):
            dst = _mkap(_br, abb_t[:], r0 * 20 + col, [[20, brows], [1, 10]])
            nc.scalar.copy(
                dst, mm[:, 0:bcols].rearrange("p (r c) -> p r c", c=10))
        for mm, col in ((mma, 0), (mmb, 2)):
            dst = _mkap(_br, abe[:], r0 * 4 + col, [[4, brows], [1, 2]])
            src = _mkap(_br, mm[:, 0:bcols], 0, [[10, brows], [9, 2]])
            nc.scalar.copy(dst, src)
        r0 += brows
    return abb_t, abe


def _emit_back(nc, pools, front, yv, R, Rd):
    """Max, multiply-trees, normalize, store."""
    import bass_rust as _br
    import concourse.mybir as mybir
    from concourse.mybir import AluOpType

    F32 = mybir.dt.float32
    BF16 = mybir.dt.bfloat16
    io, ptp, abf, abb, pps, lptp, e4p, sm, tpp, mmp = pools
    abb_t, abe = front
    Rp = R - Rd

    def ap(tile_ap, off, dims):
        return _mkap(_br, tile_ap, off, dims)

    lpt = lptp.tile([P, R, 19], F32, tag="lpt")
    pp = pps.tile([P, R, PP_W], BF16, tag="pp")
    A, Bo = 0, 10
    for k in MID_KS:
        c = k + 1
        nc.vector.tensor_tensor(
            ap(pp[:], PP_SLOT[k], [[PP_W, R], [PP_MIR, 2], [1, c]]),
            ap(abb_t[:], A, [[20, R], [9 - k, 2], [1, c]]),
            ap(abb_t[:], Bo + k, [[20, R], [9 - k, 2], [-1, c]]),
            AluOpType.max)
    nc.vector.tensor_tensor(
        ap(pp[:], PP_SLOT[9], [[PP_W, R], [1, 10]]),
        ap(abb_t[:], A, [[20, R], [1, 10]]),
        ap(abb_t[:], Bo + 9, [[20, R], [-1, 10]]),
        AluOpType.max)
    # edge bins 0/18 in fp32 (abe cols: A0 A9 B0 B9)
    nc.vector.tensor_tensor(
        ap(lpt[:], 0, [[19, R], [18, 2]]),
        ap(abe[:], 0, [[4, R], [1, 2]]),
        ap(abe[:], 2, [[4, R], [1, 2]]),
        AluOpType.max)
    # multiply-trees. DVE rows [0, Rd): levels merged across adjacent
    # pairs (uniform slot layout; in1 offsets align at stride 11 where ce
    # increments). Pool rows [Rd, R): per-bin 2-dim APs (Pool ucode has
    # mult but not max/min). Final level writes fp32 into lpt.
    if Rd > 0:
        M2 = [[PP_MIR, 2]]
        mul = AluOpType.mult
        def dv(o_off, o_dims, i_off, i_dims):
            nc.vector.tensor_tensor(ap(pp[:], o_off, [[PP_W, Rd]] + o_dims),
                                    ap(pp[:], o_off, [[PP_W, Rd]] + o_dims),
                                    ap(pp[:], i_off, [[PP_W, Rd]] + i_dims),
                                    mul)
        # L1
        dv(10, M2 + [[1, 1]], 12, M2 + [[1, 1]])                      # c3
        dv(20, M2 + [[10, 2], [1, 2]], 22, M2 + [[11, 2], [1, 2]])    # c4,c5
        dv(40, M2 + [[10, 2], [1, 3]], 43, M2 + [[11, 2], [1, 3]])    # c6,c7
        dv(60, M2 + [[10, 2], [1, 4]], 64, M2 + [[11, 2], [1, 4]])    # c8,c9
        dv(80, [[1, 5]], 85, [[1, 5]])                                # c10
        # L2
        dv(30, M2 + [[10, 2], [1, 1]], 32, M2 + [[10, 2], [1, 1]])    # c5,c6
        dv(50, M2 + [[10, 2], [1, 2]], 52, M2 + [[10, 2], [1, 2]])    # c7,c8
        dv(70, M2 + [[1, 2]], 73, M2 + [[1, 2]])                      # c9
        dv(80, [[1, 2]], 83, [[1, 2]])                                # c10
        # L3
        dv(70, M2 + [[1, 1]], 72, M2 + [[1, 1]])                      # c9
        dv(80, [[1, 1]], 82, [[1, 1]])                                # c10
        # finals -> lpt (fp32)
        for k in MID_KS:
            nc.vector.tensor_tensor(
                ap(lpt[:], k, [[19, Rd], [18 - 2 * k, 2], [1, 1]]),
                ap(pp[:], PP_SLOT[k], [[PP_W, Rd]] + M2 + [[1, 1]]),
                ap(pp[:], PP_SLOT[k] + 1, [[PP_W, Rd]] + M2 + [[1, 1]]),
                mul)
        nc.vector.tensor_tensor(
            ap(lpt[:], 9, [[19, Rd], [1, 1]]),
            ap(pp[:], 80, [[PP_W, Rd], [1, 1]]),
            ap(pp[:], 81, [[PP_W, Rd], [1, 1]]), mul)
    if Rp > 0:
        for k in MID_KS + [9]:
            c = k + 1 if k != 9 else 10
            sides = (((0, k), (1, 18 - k)) if k != 9 else ((0, 9),))
            for psel, kk in sides:
                pb = Rd * PP_W + PP_SLOT[k] + psel * PP_MIR
                c0 = c
                while c0 > 2:
                    fl, ce = c0 // 2, c0 - c0 // 2
                    nc.gpsimd.tensor_tensor(
                        ap(pp[:], pb, [[PP_W, Rp], [1, fl]]),
                        ap(pp[:], pb, [[PP_W, Rp], [1, fl]]),
                        ap(pp[:], pb + ce, [[PP_W, Rp], [1, fl]]),
                        AluOpType.mult)
                    c0 = ce
                nc.gpsimd.tensor_tensor(
                    ap(lpt[:], Rd * 19 + kk, [[19, Rp], [1, 1]]),
                    ap(pp[:], pb, [[PP_W, Rp], [1, 1]]),
                    ap(pp[:], pb + 1, [[PP_W, Rp], [1, 1]]),
                    AluOpType.mult)

    # normalize: y = (1 - P) / (19 + 1e-9 - sum P); sum P as Pool add-tree
    st_ = sm.tile([P, R], F32, tag="S")
    sacc = sm.tile([P, R, 9], F32, tag="sacc")
    nc.gpsimd.tensor_tensor(sacc[:], lpt[:, :, 0:9], lpt[:, :, 10:19],
                            AluOpType.add)
    nc.gpsimd.tensor_tensor(sacc[:, :, 0:4], sacc[:, :, 0:4],
                            sacc[:, :, 5:9], AluOpType.add)
    nc.gpsimd.tensor_tensor(sacc[:, :, 0:2], sacc[:, :, 0:2],
                            sacc[:, :, 2:4], AluOpType.add)
    nc.gpsimd.tensor_tensor(sacc[:, :, 0:1], sacc[:, :, 0:1],
                            sacc[:, :, 1:2], AluOpType.add)
    nc.gpsimd.tensor_tensor(sacc[:, :, 4:5], sacc[:, :, 4:5],
                            lpt[:, :, 9:10], AluOpType.add)
    nc.gpsimd.tensor_tensor(st_[:].unsqueeze(2), sacc[:, :, 0:1],
                            sacc[:, :, 4:5], AluOpType.add)
    nc.gpsimd.tensor_scalar(st_[:], st_[:], -1.0, 19.0 + 1e-9,
                            AluOpType.mult, AluOpType.add)
    rt = sm.tile([P, R], F32, tag="r")
    nc.vector.reciprocal(rt[:], st_[:])
    nc.scalar.activation(
        lpt[:].rearrange("p r k -> p (r k)"),
        lpt[:].rearrange("p r k -> p (r k)"),
        mybir.ActivationFunctionType.Copy, bias=1.0, scale=-1.0)
    nc.gpsimd.tensor_tensor(
        lpt[:], lpt[:],
        rt[:].unsqueeze(2).broadcast_to((P, R, 19)),
        AluOpType.mult)
    nc.sync.dma_start(yv, lpt[:].rearrange("p r k -> p (r k)"))


def _build_core(rows_total, sched, rd_map):
    import concourse.mybir as mybir
    from concourse.bacc import Bacc
    from concourse.tile import TileContext

    F32 = mybir.dt.float32

    nc = Bacc()
    p1d = nc.dram_tensor("p1", [rows_total, 10], F32, kind="ExternalInput")
    p2d = nc.dram_tensor("p2", [rows_total, 10], F32, kind="ExternalInput")
    v1d = nc.dram_tensor("v1b", [120, 120], F32, kind="ExternalInput")
    v2d = nc.dram_tensor("v2b", [120, 120], F32, kind="ExternalInput")
    idd = nc.dram_tensor("ident", [128, 128], F32, kind="ExternalInput")
    yd = nc.dram_tensor("y", [rows_total, 19], F32, kind="ExternalOutput")

    with TileContext(nc) as tc:
        with (
            tc.tile_pool(name="const", bufs=1) as cpool,
            tc.tile_pool(name="io", bufs=3) as io,
            tc.tile_pool(name="pt", bufs=2) as ptp,
            tc.tile_pool(name="abf", bufs=3) as abf,
            tc.tile_pool(name="abb", bufs=3) as abb,
            tc.tile_pool(name="pps", bufs=2) as pps,
            tc.tile_pool(name="lpt", bufs=3) as lptp,
            tc.tile_pool(name="e4", bufs=2) as e4p,
            tc.tile_pool(name="sm", bufs=2) as sm,
            tc.tile_pool(name="tp", bufs=2, space="PSUM") as tpp,
            tc.tile_pool(name="mm", bufs=2, space="PSUM") as mmp,
        ):
            v1t = cpool.tile([120, 120], F32)
            v2t = cpool.tile([120, 120], F32)
            idt = cpool.tile([128, 128], F32)
            nc.sync.dma_start(idt[:], idd[:])
            nc.sync.dma_start(v1t[:], v1d[:])
            nc.sync.dma_start(v2t[:], v2d[:])
            pools = (io, ptp, abf, abb, pps, lptp, e4p, sm, tpp, mmp)
            consts = (v1t, v2t, idt)

            n = len(sched)
            loads, fronts = {}, {}
            for i in range(min(2, n)):
                loads[i] = _emit_loads(nc, io, p1d, p2d, *sched[i])
            if n:
                fronts[0] = _emit_front(nc, pools, consts, sched[0][1],
                                        loads.pop(0))
            for i in range(n):
                if i + 2 < n:
                    loads[i + 2] = _emit_loads(nc, io, p1d, p2d,
                                               *sched[i + 2])
                if i + 1 < n:
                    fronts[i + 1] = _emit_front(nc, pools, consts,
                                                sched[i + 1][1],
                                                loads.pop(i + 1))
                row0, R = sched[i]
                nrows = P * R
                yv = yd[row0:row0 + nrows, :].rearrange(
                    "(p r) k -> p (r k)", p=P)
                _emit_back(nc, pools, fronts.pop(i), yv, R, rd_map[R])

    nc.finalize()
    return nc


RD_MAP = {128: 32, 108: 27, 96: 24, 64: 16, 48: 12, 36: 9,
          32: 8, 24: 6, 16: 4, 12: 3, 8: 2}


def _build_nc(nt=NT, reps=1):
    sched = [s for _ in range(reps) for s in _schedule(nt)]
    return _build_core(RPC, sched, RD_MAP)


def _host_consts(W1, W2):
    def mmn(W):
        W = W.astype(np.float32)
        lo = W.min(1, keepdims=True)
        hi = W.max(1, keepdims=True)
        return (W - lo) / (hi - lo + np.float32(1e-8))

    eye12 = np.eye(12, dtype=np.float32)
    v1b = np.kron(eye12, (np.float32(1.0) - mmn(W1))).astype(np.float32)
    v2b = np.kron(eye12, (np.float32(1.0) - mmn(W2))).astype(np.float32)
    ident = np.eye(128, dtype=np.float32)
    return v1b, v2b, ident


def kernel(p1, p2, W1, W2, mask=None, **_unused):
    from concourse.bass_utils import run_bass_kernel_spmd

    if 'nc' not in _CACHED:
        _CACHED['nc'] = _build_nc()
    nc = _CACHED['nc']

    v1b, v2b, ident = _host_consts(W1, W2)
    p1 = np.ascontiguousarray(p1, dtype=np.float32)
    p2 = np.ascontiguousarray(p2, dtype=np.float32)

    in_maps = []
    for c in range(N_CORES):
        sl = slice(c * RPC, (c + 1) * RPC)
        in_maps.append({
            "p1": p1[sl], "p2": p2[sl],
            "v1b": v1b, "v2b": v2b, "ident": ident,
        })
    res = run_bass_kernel_spmd(nc, in_maps, list(range(N_CORES)))
    out = np.concatenate([res.results[c]["y"] for c in range(N_CORES)], axis=0)
    return out.astype(np.float32)


def _numpy_ref(p1, p2, W1, W2):
    def mmn(W):
        lo = W.min(1, keepdims=True); hi = W.max(1, keepdims=True)
        return (W - lo) / (hi - lo + np.float32(1e-8))
    A = p1 @ (1.0 - mmn(W1))
    Bv = p2 @ (1.0 - mmn(W2))
    st = np.maximum(A[:, :, None], Bv[:, None, :])
    Pk = np.ones((p1.shape[0], 19), np.float64)
    for i in range(10):
        for j in range(10):
            Pk[:, i + j] *= st[:, i, j]
    y = 1.0 - Pk
    return (y / (y.sum(1, keepdims=True) + 1e-9)).astype(np.float32)


def _simcheck():
    """CoreSim-exec validation on a small schedule."""
    from concourse.bass_interp import CoreSim
    rows = P * 16
    nc = _build_core(rows, [(0, 8), (P * 8, 8)], RD_MAP)
    rng = np.random.default_rng(1)
    p1 = rng.random((rows, 10), dtype=np.float32)
    p1 /= p1.sum(1, keepdims=True)
    p2 = rng.random((rows, 10), dtype=np.float32)
    p2 /= p2.sum(1, keepdims=True)
    W1 = rng.random((10, 10), dtype=np.float32)
    W2 = rng.random((10, 10), dtype=np.float32)
    v1b, v2b, ident = _host_consts(W1, W2)

    vals = {"p1": p1, "p2": p2, "v1b": v1b, "v2b": v2b, "ident": ident}
    bufs = {}
    for alloc in nc.m.functions[0].allocations:
        if hasattr(alloc, 'memorylocations') and alloc.memorylocations:
            for mem in alloc.memorylocations:
                if mem.name in vals:
                    a = np.ascontiguousarray(vals[mem.name])
                    bufs[mem.name] = a.view(np.uint8).reshape(tuple(mem.dims))
                elif mem.name == "y":
                    bufs["y"] = np.zeros(tuple(mem.dims), np.uint8)
    sim = CoreSim(nc, preallocated_bufs=bufs)
    sim.simulate()
    y = bufs["y"].view(np.float32).reshape(rows, 19)
    ref = _numpy_ref(p1, p2, W1, W2)
    rel = np.abs(y - ref) / np.maximum(np.abs(ref), 1e-12)
    print(f"simcheck: rel err max {rel.max():.3e}  sim time {sim.time:.0f} ns")
    assert rel.max() < 2e-2, "simcheck FAILED"
    print("simcheck PASSED")


if __name__ == "__main__":
    if "--simcheck" in sys.argv:
        _simcheck()
    else:
        rng = np.random.default_rng(0)
        p1 = rng.random((B, 10), dtype=np.float32)
        p1 /= p1.sum(1, keepdims=True)
        p2 = rng.random((B, 10), dtype=np.float32)
        p2 /= p2.sum(1, keepdims=True)
        W1 = rng.random((10, 10), dtype=np.float32)
        W2 = rng.random((10, 10), dtype=np.float32)
        y = kernel(p1, p2, W1, W2)
        print("kernel ran, y shape", y.shape, "sum", float(y.sum()))


# revision 37
# speedup vs baseline: 2.6504x; 1.0021x over previous
"""Trainium2 Bass kernel for nn_BaconAdditionReasoner (segment_reduce).

Math (per row b of 1M):
  a = p1 @ minmax(W1); b = p2 @ minmax(W2)           # [10] each
  s_ij = min(a_i, b_j); one_minus = 1 - clip(s)       # [10,10]
  y_k  = 1 - prod_{i+j=k} one_minus_ij                # 19 anti-diag bins
  y    = y / (sum_k y_k + 1e-9)

Kernel formulation (linear space, no Ln/Exp round-trip):
  A = p1 @ (1 - minmax(W1)); B = p2 @ (1 - minmax(W2))   # = one_minus rows
  one_minus_ij = max(A_i, B_j)   [clip never fires: A,B in (0.11, 0.84)]
  P_k = prod over anti-diagonal; y = (1 - P) / (19 + 1e-9 - sum P)

Engine split per tile (R rows/partition, rows on 128 partitions):
  PE    : 12-row-packed transposes + matmuls vs kron(I_12, V)
  ACT   : PSUM->SBUF copies (4-group-batched transpose spill and A|B
          copies), u = 1 - P
  DVE   : bf16 pair-packed max over mirror-bin pairs for bins 2..16
          (all strides +-1 -> 2x_1p double rate); fp32 edge bins
          0/1/17/18 (error amplification P/(1-P) is up to 4.1 there,
          <=0.54 for mid bins, so bf16 is only safe for the latter);
          bf16 multiply-trees for rows [0, Rd); reciprocal
  Pool  : multiply-trees for rows [Rd, R) via per-bin 2-dim APs (Pool
          ucode implements add/sub/mult/copy only -- no max/min/PSUM),
          the 19-bin add-tree for sum P, and y = u * r

Tiles are emitted software-pipelined (frontend of tile i+1 before the
backend of tile i, input DMAs two tiles ahead) so the in-order engine
queues interleave ready work instead of stalling on cross-engine waits.

Sharding: pure data parallel over 8 cores, 131072 rows each.
"""
import sys

if '/opt/trn_rl_repo' not in sys.path:
    sys.path.insert(0, '/opt/trn_rl_repo')

import numpy as np

B = 1048576
N_CORES = 8
RPC = B // N_CORES          # 131072 rows per core
P = 128                     # partitions
NT = 16                     # work units of 8192 rows (bench scaling)

# Uniform-slot layout for bins (k, 18-k), k=1..8 (cnt c=k+1), plus bin 9:
# bin k lives at slot (k-1)*10, its mirror at +90, bin 9 at 80. The
# constant slot/mirror strides let multiply-tree levels merge across
# adjacent pairs (equal fl; the ce offsets line up as stride-11 APs).
# Bins 1/17 ride the bf16 path: their amplification P/(1-P) is <= 0.98 so
# the ~0.8% worst-case bf16 error stays well under the 2e-2 gate; only
# bins 0/18 (ratio up to 4.1) need fp32.
MID_KS = [1, 2, 3, 4, 5, 6, 7, 8]
PP_SLOT = {k: (k - 1) * 10 for k in MID_KS}
PP_SLOT[9] = 80
PP_MIR = 90
PP_W = 180


def _groups_for(r):
    g = [12] * (r // 12)
    if r % 12:
        g.append(r % 12)
    return g


def _batches_for(r):
    """<=4 transpose groups (<=480 PSUM columns) per PSUM tile / copy."""
    gs = _groups_for(r)
    out, cur = [], []
    for g in gs:
        cur.append(g)
        if len(cur) == 4:
            out.append(cur)
            cur = []
    if cur:
        out.append(cur)
    return out


def _schedule(nt):
    rows = nt * P * 64
    rpp = rows // P
    # ramp tile sizes up at the start and down at the end: shortens the
    # pipeline fill (engines wait on tile 0's serial frontend) and the
    # drain (the last tile's serial backend tail)
    ramp_up, ramp_dn = [12, 12, 24, 48, 96], [96, 48, 36, 12]
    out, row0 = [], 0
    if rpp >= sum(ramp_up) + sum(ramp_dn) + 128:
        mid = rpp - sum(ramp_up) - sum(ramp_dn)
        plan = list(ramp_up) + [128] * (mid // 128)
        if mid % 128:
            plan.append(mid % 128)
        plan += ramp_dn
    else:
        plan = []
        left = rpp
        while left > 0:
            r = min(128, left)
            plan.append(r); left -= r
    for r in plan:
        out.append((row0, r)); row0 += P * r
    assert row0 == rows
    return out

_CACHED = {}


def _mkap(br, tile_ap, off, dims):
    return br.AP(tensor=tile_ap.tensor, offset=tile_ap.offset + off,
                 ap=[list(tile_ap.ap[0])] + [list(d) for d in dims])


def _emit_loads(nc, io, p1d, p2d, row0, R):
    import concourse.mybir as mybir
    F32 = mybir.dt.float32
    nrows = P * R
    p1v = p1d[row0:row0 + nrows, :].rearrange("(p r) c -> p (r c)", p=P)
    p2v = p2d[row0:row0 + nrows, :].rearrange("(p r) c -> p (r c)", p=P)
    p1t = io.tile([P, R * 10], F32, tag="p1t")
    p2t = io.tile([P, R * 10], F32, tag="p2t")
    nc.sync.dma_start(p1t[:], p1v)
    nc.sync.dma_start(p2t[:], p2v)
    return p1t, p2t


def _emit_front(nc, pools, consts, R, loaded):
    """Transposes, matmuls, PSUM->SBUF copies.

    Produces abb [P, R, 20] bf16 (A | B rows for the pair-packed max) and
    abe [P, R, 4] fp32 (A0 A9 B0 B9, for the fp32 edge bins 0/18)."""
    import bass_rust as _br
    import concourse.mybir as mybir

    F32 = mybir.dt.float32
    BF16 = mybir.dt.bfloat16
    io, ptp, abf, abb, pps, lptp, e4p, sm, tpp, mmp = pools
    v1t, v2t, idt = consts
    p1t, p2t = loaded

    abb_t = abb.tile([P, R, 20], BF16, tag="abb")
    abe = abf.tile([P, R, 4], F32, tag="abe")
    r0 = 0
    for batch in _batches_for(R):
        brows = sum(batch)
        bcols = brows * 10
        mma = mmp.tile([P, 480], F32, tag="mma")
        mmb = mmp.tile([P, 480], F32, tag="mmb")
        tpa = tpp.tile([120, 512], F32, tag="tpa")
        tpb = tpp.tile([120, 512], F32, tag="tpb")
        pta = ptp.tile([120, 512], F32, tag="pta")
        ptb = ptp.tile([120, 512], F32, tag="ptb")
        for src, vt, mm, tp, pt in ((p1t, v1t, mma, tpa, pta),
                                    (p2t, v2t, mmb, tpb, ptb)):
            off = 0
            full_k = 0
            for gi, gs in enumerate(batch):
                K = gs * 10
                nc.tensor.transpose(
                    tp[0:K, gi * 128:(gi + 1) * 128],
                    src[:, (r0 * 10 + off):(r0 * 10 + off + K)], idt[:])
                off += K
                if K == 120:
                    full_k += 1
            # batched PSUM->SBUF copy: full 120-row groups in one shot,
            # the ragged tail group (if any) separately
            if full_k:
                nc.scalar.copy(pt[:, 0:full_k * 128], tp[:, 0:full_k * 128])
            if full_k < len(batch):
                K = batch[-1] * 10
                nc.scalar.copy(pt[0:K, full_k * 128:full_k * 128 + 128],
                               tp[0:K, full_k * 128:full_k * 128 + 128])
            off = 0
            for gi, gs in enumerate(batch):
                K = gs * 10
                nc.tensor.matmul(mm[:, off:off + K],
                                 pt[0:K, gi * 128:(gi + 1) * 128],
                                 vt[0:K, 0:K], start=True, stop=True)
                off += K
        for mm, col in ((mma, 0), (mmb, 10)):
            dst = _mkap(_br, abb_t[:], r0 * 20 + col, [[20, brows], [1, 10]])
            nc.scalar.copy(
                dst, mm[:, 0:bcols].rearrange("p (r c) -> p r c", c=10))
        for mm, col in ((mma, 0), (mmb, 2)):
            dst = _mkap(_br, abe[:], r0 * 4 + col, [[4, brows], [1, 2]])
            src = _mkap(_br, mm[:, 0:bcols], 0, [[10, brows], [9, 2]])
            nc.scalar.copy(dst, src)
        r0 += brows
    return abb_t, abe


def _emit_back(nc, pools, front, yv, R, Rd):
    """Max, multiply-trees, normalize, store."""
    import bass_rust as _br
    import concourse.mybir as mybir
    from concourse.mybir import AluOpType

    F32 = mybir.dt.float32
    BF16 = mybir.dt.bfloat16
    io, ptp, abf, abb, pps, lptp, e4p, sm, tpp, mmp = pools
    abb_t, abe = front
    Rp = R - Rd

    def ap(tile_ap, off, dims):
        return _mkap(_br, tile_ap, off, dims)

    lpt = lptp.tile([P, R, 19], F32, tag="lpt")
    pp = pps.tile([P, R, PP_W], BF16, tag="pp")
    A, Bo = 0, 10
    for k in MID_KS:
        c = k + 1
        nc.vector.tensor_tensor(
            ap(pp[:], PP_SLOT[k], [[PP_W, R], [PP_MIR, 2], [1, c]]),
            ap(abb_t[:], A, [[20, R], [9 - k, 2], [1, c]]),
            ap(abb_t[:], Bo + k, [[20, R], [9 - k, 2], [-1, c]]),
            AluOpType.max)
    nc.vector.tensor_tensor(
        ap(pp[:], PP_SLOT[9], [[PP_W, R], [1, 10]]),
        ap(abb_t[:], A, [[20, R], [1, 10]]),
        ap(abb_t[:], Bo + 9, [[20, R], [-1, 10]]),
        AluOpType.max)
    # edge bins 0/18 in fp32 (abe cols: A0 A9 B0 B9)
    nc.vector.tensor_tensor(
        ap(lpt[:], 0, [[19, R], [18, 2]]),
        ap(abe[:], 0, [[4, R], [1, 2]]),
        ap(abe[:], 2, [[4, R], [1, 2]]),
        AluOpType.max)
    # multiply-trees. DVE rows [0, Rd): levels merged across adjacent
    # pairs (uniform slot layout; in1 offsets align at stride 11 where ce
    # increments). Pool rows [Rd, R): per-bin 2-dim APs (Pool ucode has
    # mult but not max/min). Final level writes fp32 into lpt.
    if Rd > 0:
        M2 = [[PP_MIR, 2]]
        mul = AluOpType.mult
        def dv(o_off, o_dims, i_off, i_dims):
            nc.vector.tensor_tensor(ap(pp[:], o_off, [[PP_W, Rd]] + o_dims),
                                    ap(pp[:], o_off, [[PP_W, Rd]] + o_dims),
                                    ap(pp[:], i_off, [[PP_W, Rd]] + i_dims),
                                    mul)
        # L1
        dv(10, M2 + [[1, 1]], 12, M2 + [[1, 1]])                      # c3
        dv(20, M2 + [[10, 2], [1, 2]], 22, M2 + [[11, 2], [1, 2]])    # c4,c5
        dv(40, M2 + [[10, 2], [1, 3]], 43, M2 + [[11, 2], [1, 3]])    # c6,c7
        dv(60, M2 + [[10, 2], [1, 4]], 64, M2 + [[11, 2], [1, 4]])    # c8,c9
        dv(80, [[1, 5]], 85, [[1, 5]])                                # c10
        # L2
        dv(30, M2 + [[10, 2], [1, 1]], 32, M2 + [[10, 2], [1, 1]])    # c5,c6
        dv(50, M2 + [[10, 2], [1, 2]], 52, M2 + [[10, 2], [1, 2]])    # c7,c8
        dv(70, M2 + [[1, 2]], 73, M2 + [[1, 2]])                      # c9
        dv(80, [[1, 2]], 83, [[1, 2]])                                # c10
        # L3
        dv(70, M2 + [[1, 1]], 72, M2 + [[1, 1]])                      # c9
        dv(80, [[1, 1]], 82, [[1, 1]])                                # c10
        # finals -> lpt (fp32)
        for k in MID_KS:
            nc.vector.tensor_tensor(
                ap(lpt[:], k, [[19, Rd], [18 - 2 * k, 2], [1, 1]]),
                ap(pp[:], PP_SLOT[k], [[PP_W, Rd]] + M2 + [[1, 1]]),
                ap(pp[:], PP_SLOT[k] + 1, [[PP_W, Rd]] + M2 + [[1, 1]]),
                mul)
        nc.vector.tensor_tensor(
            ap(lpt[:], 9, [[19, Rd], [1, 1]]),
            ap(pp[:], 80, [[PP_W, Rd], [1, 1]]),
            ap(pp[:], 81, [[PP_W, Rd], [1, 1]]), mul)
    if Rp > 0:
        for k in MID_KS + [9]:
            c = k + 1 if k != 9 else 10
            sides = (((0, k), (1, 18 - k)) if k != 9 else ((0, 9),))
            for psel, kk in sides:
                pb = Rd * PP_W + PP_SLOT[k] + psel * PP_MIR
                c0 = c
                while c0 > 2:
                    fl, ce = c0 // 2, c0 - c0 // 2
                    nc.gpsimd.tensor_tensor(
                        ap(pp[:], pb, [[PP_W, Rp], [1, fl]]),
                        ap(pp[:], pb, [[PP_W, Rp], [1, fl]]),
                        ap(pp[:], pb + ce, [[PP_W, Rp], [1, fl]]),
                        AluOpType.mult)
                    c0 = ce
                nc.gpsimd.tensor_tensor(
                    ap(lpt[:], Rd * 19 + kk, [[19, Rp], [1, 1]]),
                    ap(pp[:], pb, [[PP_W, Rp], [1, 1]]),
                    ap(pp[:], pb + 1, [[PP_W, Rp], [1, 1]]),
                    AluOpType.mult)

    # normalize: y = (1 - P) / (19 + 1e-9 - sum P); sum P as Pool add-tree
    st_ = sm.tile([P, R], F32, tag="S")
    sacc = sm.tile([P, R, 9], F32, tag="sacc")
    nc.gpsimd.tensor_tensor(sacc[:], lpt[:, :, 0:9], lpt[:, :, 10:19],
                            AluOpType.add)
    nc.gpsimd.tensor_tensor(sacc[:, :, 0:4], sacc[:, :, 0:4],
                            sacc[:, :, 5:9], AluOpType.add)
    nc.gpsimd.tensor_tensor(sacc[:, :, 0:2], sacc[:, :, 0:2],
                            sacc[:, :, 2:4], AluOpType.add)
    nc.gpsimd.tensor_tensor(sacc[:, :, 0:1], sacc[:, :, 0:1],
                            sacc[:, :, 1:2], AluOpType.add)
    nc.gpsimd.tensor_tensor(sacc[:, :, 4:5], sacc[:, :, 4:5],
                            lpt[:, :, 9:10], AluOpType.add)
    nc.gpsimd.tensor_tensor(st_[:].unsqueeze(2), sacc[:, :, 0:1],
                            sacc[:, :, 4:5], AluOpType.add)
    nc.gpsimd.tensor_scalar(st_[:], st_[:], -1.0, 19.0 + 1e-9,
                            AluOpType.mult, AluOpType.add)
    rt = sm.tile([P, R], F32, tag="r")
    nc.vector.reciprocal(rt[:], st_[:])
    nc.scalar.activation(
        lpt[:].rearrange("p r k -> p (r k)"),
        lpt[:].rearrange("p r k -> p (r k)"),
        mybir.ActivationFunctionType.Copy, bias=1.0, scale=-1.0)
    nc.gpsimd.tensor_tensor(
        lpt[:], lpt[:],
        rt[:].unsqueeze(2).broadcast_to((P, R, 19)),
        AluOpType.mult)
    nc.sync.dma_start(yv, lpt[:].rearrange("p r k -> p (r k)"))


def _build_core(rows_total, sched, rd_map):
    import concourse.mybir as mybir
    from concourse.bacc import Bacc
    from concourse.tile import TileContext

    F32 = mybir.dt.float32

    nc = Bacc()
    p1d = nc.dram_tensor("p1", [rows_total, 10], F32, kind="ExternalInput")
    p2d = nc.dram_tensor("p2", [rows_total, 10], F32, kind="ExternalInput")
    v1d = nc.dram_tensor("v1b", [120, 120], F32, kind="ExternalInput")
    v2d = nc.dram_tensor("v2b", [120, 120], F32, kind="ExternalInput")
    idd = nc.dram_tensor("ident", [128, 128], F32, kind="ExternalInput")
    yd = nc.dram_tensor("y", [rows_total, 19], F32, kind="ExternalOutput")

    with TileContext(nc) as tc:
        with (
            tc.tile_pool(name="const", bufs=1) as cpool,
            tc.tile_pool(name="io", bufs=3) as io,
            tc.tile_pool(name="pt", bufs=2) as ptp,
            tc.tile_pool(name="abf", bufs=3) as abf,
            tc.tile_pool(name="abb", bufs=3) as abb,
            tc.tile_pool(name="pps", bufs=2) as pps,
            tc.tile_pool(name="lpt", bufs=3) as lptp,
            tc.tile_pool(name="e4", bufs=2) as e4p,
            tc.tile_pool(name="sm", bufs=2) as sm,
            tc.tile_pool(name="tp", bufs=2, space="PSUM") as tpp,
            tc.tile_pool(name="mm", bufs=2, space="PSUM") as mmp,
        ):
            v1t = cpool.tile([120, 120], F32)
            v2t = cpool.tile([120, 120], F32)
            idt = cpool.tile([128, 128], F32)
            nc.sync.dma_start(idt[:], idd[:])
            nc.sync.dma_start(v1t[:], v1d[:])
            nc.sync.dma_start(v2t[:], v2d[:])
            pools = (io, ptp, abf, abb, pps, lptp, e4p, sm, tpp, mmp)
            consts = (v1t, v2t, idt)

            n = len(sched)
            loads, fronts = {}, {}
            for i in range(min(2, n)):
                loads[i] = _emit_loads(nc, io, p1d, p2d, *sched[i])
            if n:
                fronts[0] = _emit_front(nc, pools, consts, sched[0][1],
                                        loads.pop(0))
            for i in range(n):
                if i + 2 < n:
                    loads[i + 2] = _emit_loads(nc, io, p1d, p2d,
                                               *sched[i + 2])
                if i + 1 < n:
                    fronts[i + 1] = _emit_front(nc, pools, consts,
                                                sched[i + 1][1],
                                                loads.pop(i + 1))
                row0, R = sched[i]
                nrows = P * R
                yv = yd[row0:row0 + nrows, :].rearrange(
                    "(p r) k -> p (r k)", p=P)
                _emit_back(nc, pools, fronts.pop(i), yv, R, rd_map[R])

    nc.finalize()
    return nc


RD_MAP = {128: 32, 108: 24, 96: 21, 64: 14, 48: 10, 36: 8,
          32: 7, 24: 5, 16: 3, 12: 2, 8: 2}


def _build_nc(nt=NT, reps=1):
    sched = [s for _ in range(reps) for s in _schedule(nt)]
    return _build_core(RPC, sched, RD_MAP)


def _host_consts(W1, W2):
    def mmn(W):
        W = W.astype(np.float32)
        lo = W.min(1, keepdims=True)
        hi = W.max(1, keepdims=True)
        return (W - lo) / (hi - lo + np.float32(1e-8))

    eye12 = np.eye(12, dtype=np.float32)
    v1b = np.kron(eye12, (np.float32(1.0) - mmn(W1))).astype(np.float32)
    v2b = np.kron(eye12, (np.float32(1.0) - mmn(W2))).astype(np.float32)
    ident = np.eye(128, dtype=np.float32)
    return v1b, v2b, ident


def kernel(p1, p2, W1, W2, mask=None, **_unused):
    from concourse.bass_utils import run_bass_kernel_spmd

    if 'nc' not in _CACHED:
        _CACHED['nc'] = _build_nc()
    nc = _CACHED['nc']

    v1b, v2b, ident = _host_consts(W1, W2)
    p1 = np.ascontiguousarray(p1, dtype=np.float32)
    p2 = np.ascontiguousarray(p2, dtype=np.float32)

    in_maps = []
    for c in range(N_CORES):
        sl = slice(c * RPC, (c + 1) * RPC)
        in_maps.append({
            "p1": p1[sl], "p2": p2[sl],
            "v1b": v1b, "v2b": v2b, "ident": ident,
        })
    res = run_bass_kernel_spmd(nc, in_maps, list(range(N_CORES)))
    out = np.concatenate([res.results[c]["y"] for c in range(N_CORES)], axis=0)
    return out.astype(np.float32)


def _numpy_ref(p1, p2, W1, W2):
    def mmn(W):
        lo = W.min(1, keepdims=True); hi = W.max(1, keepdims=True)
        return (W - lo) / (hi - lo + np.float32(1e-8))
    A = p1 @ (1.0 - mmn(W1))
    Bv = p2 @ (1.0 - mmn(W2))
    st = np.maximum(A[:, :, None], Bv[:, None, :])
    Pk = np.ones((p1.shape[0], 19), np.float64)
    for i in range(10):
        for j in range(10):
            Pk[:, i + j] *= st[:, i, j]
    y = 1.0 - Pk
    return (y / (y.sum(1, keepdims=True) + 1e-9)).astype(np.float32)


def _simcheck():
    """CoreSim-exec validation on a small schedule."""
    from concourse.bass_interp import CoreSim
    rows = P * 16
    nc = _build_core(rows, [(0, 8), (P * 8, 8)], RD_MAP)
    rng = np.random.default_rng(1)
    p1 = rng.random((rows, 10), dtype=np.float32)
    p1 /= p1.sum(1, keepdims=True)
    p2 = rng.random((rows, 10), dtype=np.float32)
    p2 /= p2.sum(1, keepdims=True)
    W1 = rng.random((10, 10), dtype=np.float32)
    W2 = rng.random((10, 10), dtype=np.float32)
    v1b, v2b, ident = _host_consts(W1, W2)

    vals = {"p1": p1, "p2": p2, "v1b": v1b, "v2b": v2b, "ident": ident}
    bufs = {}
    for alloc in nc.m.functions[0].allocations:
        if hasattr(alloc, 'memorylocations') and alloc.memorylocations:
            for mem in alloc.memorylocations:
                if mem.name in vals:
                    a = np.ascontiguousarray(vals[mem.name])
                    bufs[mem.name] = a.view(np.uint8).reshape(tuple(mem.dims))
                elif mem.name == "y":
                    bufs["y"] = np.zeros(tuple(mem.dims), np.uint8)
    sim = CoreSim(nc, preallocated_bufs=bufs)
    sim.simulate()
    y = bufs["y"].view(np.float32).reshape(rows, 19)
    ref = _numpy_ref(p1, p2, W1, W2)
    rel = np.abs(y - ref) / np.maximum(np.abs(ref), 1e-12)
    print(f"simcheck: rel err max {rel.max():.3e}  sim time {sim.time:.0f} ns")
    assert rel.max() < 2e-2, "simcheck FAILED"
    print("simcheck PASSED")


if __name__ == "__main__":
    if "--simcheck" in sys.argv:
        _simcheck()
    else:
        rng = np.random.default_rng(0)
        p1 = rng.random((B, 10), dtype=np.float32)
        p1 /= p1.sum(1, keepdims=True)
        p2 = rng.random((B, 10), dtype=np.float32)
        p2 /= p2.sum(1, keepdims=True)
        W1 = rng.random((10, 10), dtype=np.float32)
        W2 = rng.random((10, 10), dtype=np.float32)
        y = kernel(p1, p2, W1, W2)
        print("kernel ran, y shape", y.shape, "sum", float(y.sum()))


# revision 38
# speedup vs baseline: 2.6517x; 1.0005x over previous
"""Trainium2 Bass kernel for nn_BaconAdditionReasoner (segment_reduce).

Math (per row b of 1M):
  a = p1 @ minmax(W1); b = p2 @ minmax(W2)           # [10] each
  s_ij = min(a_i, b_j); one_minus = 1 - clip(s)       # [10,10]
  y_k  = 1 - prod_{i+j=k} one_minus_ij                # 19 anti-diag bins
  y    = y / (sum_k y_k + 1e-9)

Kernel formulation (linear space, no Ln/Exp round-trip):
  A = p1 @ (1 - minmax(W1)); B = p2 @ (1 - minmax(W2))   # = one_minus rows
  one_minus_ij = max(A_i, B_j)   [clip never fires: A,B in (0.11, 0.84)]
  P_k = prod over anti-diagonal; y = (1 - P) / (19 + 1e-9 - sum P)

Engine split per tile (R rows/partition, rows on 128 partitions):
  PE    : 12-row-packed transposes + matmuls vs kron(I_12, V)
  ACT   : PSUM->SBUF copies (4-group-batched transpose spill and A|B
          copies), u = 1 - P
  DVE   : bf16 pair-packed max over mirror-bin pairs for bins 2..16
          (all strides +-1 -> 2x_1p double rate); fp32 edge bins
          0/1/17/18 (error amplification P/(1-P) is up to 4.1 there,
          <=0.54 for mid bins, so bf16 is only safe for the latter);
          bf16 multiply-trees for rows [0, Rd); reciprocal
  Pool  : multiply-trees for rows [Rd, R) via per-bin 2-dim APs (Pool
          ucode implements add/sub/mult/copy only -- no max/min/PSUM),
          the 19-bin add-tree for sum P, and y = u * r

Tiles are emitted software-pipelined (frontend of tile i+1 before the
backend of tile i, input DMAs two tiles ahead) so the in-order engine
queues interleave ready work instead of stalling on cross-engine waits.

Sharding: pure data parallel over 8 cores, 131072 rows each.
"""
import sys

if '/opt/trn_rl_repo' not in sys.path:
    sys.path.insert(0, '/opt/trn_rl_repo')

import numpy as np

B = 1048576
N_CORES = 8
RPC = B // N_CORES          # 131072 rows per core
P = 128                     # partitions
NT = 16                     # work units of 8192 rows (bench scaling)

# Uniform-slot layout for bins (k, 18-k), k=1..8 (cnt c=k+1), plus bin 9:
# bin k lives at slot (k-1)*10, its mirror at +90, bin 9 at 80. The
# constant slot/mirror strides let multiply-tree levels merge across
# adjacent pairs (equal fl; the ce offsets line up as stride-11 APs).
# Bins 1/17 ride the bf16 path: their amplification P/(1-P) is <= 0.98 so
# the ~0.8% worst-case bf16 error stays well under the 2e-2 gate; only
# bins 0/18 (ratio up to 4.1) need fp32.
MID_KS = [1, 2, 3, 4, 5, 6, 7, 8]
PP_SLOT = {k: (k - 1) * 10 for k in MID_KS}
PP_SLOT[9] = 80
PP_MIR = 90
PP_W = 180


def _groups_for(r):
    g = [12] * (r // 12)
    if r % 12:
        g.append(r % 12)
    return g


def _batches_for(r):
    """<=4 transpose groups (<=480 PSUM columns) per PSUM tile / copy."""
    gs = _groups_for(r)
    out, cur = [], []
    for g in gs:
        cur.append(g)
        if len(cur) == 4:
            out.append(cur)
            cur = []
    if cur:
        out.append(cur)
    return out


def _schedule(nt):
    rows = nt * P * 64
    rpp = rows // P
    # ramp tile sizes up at the start and down at the end: shortens the
    # pipeline fill (engines wait on tile 0's serial frontend) and the
    # drain (the last tile's serial backend tail)
    ramp_up, ramp_dn = [12, 12, 24, 48, 96], [96, 48, 36, 12]
    out, row0 = [], 0
    if rpp >= sum(ramp_up) + sum(ramp_dn) + 128:
        mid = rpp - sum(ramp_up) - sum(ramp_dn)
        plan = list(ramp_up) + [128] * (mid // 128)
        if mid % 128:
            plan.append(mid % 128)
        plan += ramp_dn
    else:
        plan = []
        left = rpp
        while left > 0:
            r = min(128, left)
            plan.append(r); left -= r
    for r in plan:
        out.append((row0, r)); row0 += P * r
    assert row0 == rows
    return out

_CACHED = {}


def _mkap(br, tile_ap, off, dims):
    return br.AP(tensor=tile_ap.tensor, offset=tile_ap.offset + off,
                 ap=[list(tile_ap.ap[0])] + [list(d) for d in dims])


def _emit_loads(nc, io, p1d, p2d, row0, R):
    import concourse.mybir as mybir
    F32 = mybir.dt.float32
    nrows = P * R
    p1v = p1d[row0:row0 + nrows, :].rearrange("(p r) c -> p (r c)", p=P)
    p2v = p2d[row0:row0 + nrows, :].rearrange("(p r) c -> p (r c)", p=P)
    p1t = io.tile([P, R * 10], F32, tag="p1t")
    p2t = io.tile([P, R * 10], F32, tag="p2t")
    nc.sync.dma_start(p1t[:], p1v)
    nc.sync.dma_start(p2t[:], p2v)
    return p1t, p2t


def _emit_front(nc, pools, consts, R, loaded):
    """Transposes, matmuls, PSUM->SBUF copies.

    Produces abb [P, R, 20] bf16 (A | B rows for the pair-packed max) and
    abe [P, R, 4] fp32 (A0 A9 B0 B9, for the fp32 edge bins 0/18)."""
    import bass_rust as _br
    import concourse.mybir as mybir

    F32 = mybir.dt.float32
    BF16 = mybir.dt.bfloat16
    io, ptp, abf, abb, pps, lptp, e4p, sm, tpp, mmp = pools
    v1t, v2t, idt = consts
    p1t, p2t = loaded

    abb_t = abb.tile([P, R, 20], BF16, tag="abb")
    abe = abf.tile([P, R, 4], F32, tag="abe")
    r0 = 0
    for batch in _batches_for(R):
        brows = sum(batch)
        bcols = brows * 10
        mma = mmp.tile([P, 480], F32, tag="mma")
        mmb = mmp.tile([P, 480], F32, tag="mmb")
        tpa = tpp.tile([120, 512], F32, tag="tpa")
        tpb = tpp.tile([120, 512], F32, tag="tpb")
        pta = ptp.tile([120, 512], F32, tag="pta")
        ptb = ptp.tile([120, 512], F32, tag="ptb")
        for src, vt, mm, tp, pt in ((p1t, v1t, mma, tpa, pta),
                                    (p2t, v2t, mmb, tpb, ptb)):
            off = 0
            full_k = 0
            for gi, gs in enumerate(batch):
                K = gs * 10
                nc.tensor.transpose(
                    tp[0:K, gi * 128:(gi + 1) * 128],
                    src[:, (r0 * 10 + off):(r0 * 10 + off + K)], idt[:])
                off += K
                if K == 120:
                    full_k += 1
            # batched PSUM->SBUF copy: full 120-row groups in one shot,
            # the ragged tail group (if any) separately
            if full_k:
                nc.scalar.copy(pt[:, 0:full_k * 128], tp[:, 0:full_k * 128])
            if full_k < len(batch):
                K = batch[-1] * 10
                nc.scalar.copy(pt[0:K, full_k * 128:full_k * 128 + 128],
                               tp[0:K, full_k * 128:full_k * 128 + 128])
            off = 0
            for gi, gs in enumerate(batch):
                K = gs * 10
                nc.tensor.matmul(mm[:, off:off + K],
                                 pt[0:K, gi * 128:(gi + 1) * 128],
                                 vt[0:K, 0:K], start=True, stop=True)
                off += K
        for mm, col in ((mma, 0), (mmb, 10)):
            dst = _mkap(_br, abb_t[:], r0 * 20 + col, [[20, brows], [1, 10]])
            nc.scalar.copy(
                dst, mm[:, 0:bcols].rearrange("p (r c) -> p r c", c=10))
        for mm, col in ((mma, 0), (mmb, 2)):
            dst = _mkap(_br, abe[:], r0 * 4 + col, [[4, brows], [1, 2]])
            src = _mkap(_br, mm[:, 0:bcols], 0, [[10, brows], [9, 2]])
            nc.scalar.copy(dst, src)
        r0 += brows
    return abb_t, abe


def _emit_back(nc, pools, front, yv, R, Rd):
    """Max, multiply-trees, normalize, store."""
    import bass_rust as _br
    import concourse.mybir as mybir
    from concourse.mybir import AluOpType

    F32 = mybir.dt.float32
    BF16 = mybir.dt.bfloat16
    io, ptp, abf, abb, pps, lptp, e4p, sm, tpp, mmp = pools
    abb_t, abe = front
    Rp = R - Rd

    def ap(tile_ap, off, dims):
        return _mkap(_br, tile_ap, off, dims)

    lpt = lptp.tile([P, R, 19], F32, tag="lpt")
    pp = pps.tile([P, R, PP_W], BF16, tag="pp")
    A, Bo = 0, 10
    for k in MID_KS:
        c = k + 1
        nc.vector.tensor_tensor(
            ap(pp[:], PP_SLOT[k], [[PP_W, R], [PP_MIR, 2], [1, c]]),
            ap(abb_t[:], A, [[20, R], [9 - k, 2], [1, c]]),
            ap(abb_t[:], Bo + k, [[20, R], [9 - k, 2], [-1, c]]),
            AluOpType.max)
    nc.vector.tensor_tensor(
        ap(pp[:], PP_SLOT[9], [[PP_W, R], [1, 10]]),
        ap(abb_t[:], A, [[20, R], [1, 10]]),
        ap(abb_t[:], Bo + 9, [[20, R], [-1, 10]]),
        AluOpType.max)
    # edge bins 0/18 in fp32 (abe cols: A0 A9 B0 B9)
    nc.vector.tensor_tensor(
        ap(lpt[:], 0, [[19, R], [18, 2]]),
        ap(abe[:], 0, [[4, R], [1, 2]]),
        ap(abe[:], 2, [[4, R], [1, 2]]),
        AluOpType.max)
    # multiply-trees. DVE rows [0, Rd): levels merged across adjacent
    # pairs (uniform slot layout; in1 offsets align at stride 11 where ce
    # increments). Pool rows [Rd, R): per-bin 2-dim APs (Pool ucode has
    # mult but not max/min). Final level writes fp32 into lpt.
    if Rd > 0:
        M2 = [[PP_MIR, 2]]
        mul = AluOpType.mult
        def dv(o_off, o_dims, i_off, i_dims):
            nc.vector.tensor_tensor(ap(pp[:], o_off, [[PP_W, Rd]] + o_dims),
                                    ap(pp[:], o_off, [[PP_W, Rd]] + o_dims),
                                    ap(pp[:], i_off, [[PP_W, Rd]] + i_dims),
                                    mul)
        # L1
        dv(10, M2 + [[1, 1]], 12, M2 + [[1, 1]])                      # c3
        dv(20, M2 + [[10, 2], [1, 2]], 22, M2 + [[11, 2], [1, 2]])    # c4,c5
        dv(40, M2 + [[10, 2], [1, 3]], 43, M2 + [[11, 2], [1, 3]])    # c6,c7
        dv(60, M2 + [[10, 2], [1, 4]], 64, M2 + [[11, 2], [1, 4]])    # c8,c9
        dv(80, [[1, 5]], 85, [[1, 5]])                                # c10
        # L2
        dv(30, M2 + [[10, 2], [1, 1]], 32, M2 + [[10, 2], [1, 1]])    # c5,c6
        dv(50, M2 + [[10, 2], [1, 2]], 52, M2 + [[10, 2], [1, 2]])    # c7,c8
        dv(70, M2 + [[1, 2]], 73, M2 + [[1, 2]])                      # c9
        dv(80, [[1, 2]], 83, [[1, 2]])                                # c10
        # L3
        dv(70, M2 + [[1, 1]], 72, M2 + [[1, 1]])                      # c9
        dv(80, [[1, 1]], 82, [[1, 1]])                                # c10
        # finals -> lpt (fp32)
        for k in MID_KS:
            nc.vector.tensor_tensor(
                ap(lpt[:], k, [[19, Rd], [18 - 2 * k, 2], [1, 1]]),
                ap(pp[:], PP_SLOT[k], [[PP_W, Rd]] + M2 + [[1, 1]]),
                ap(pp[:], PP_SLOT[k] + 1, [[PP_W, Rd]] + M2 + [[1, 1]]),
                mul)
        nc.vector.tensor_tensor(
            ap(lpt[:], 9, [[19, Rd], [1, 1]]),
            ap(pp[:], 80, [[PP_W, Rd], [1, 1]]),
            ap(pp[:], 81, [[PP_W, Rd], [1, 1]]), mul)
    if Rp > 0:
        for k in MID_KS + [9]:
            c = k + 1 if k != 9 else 10
            sides = (((0, k), (1, 18 - k)) if k != 9 else ((0, 9),))
            for psel, kk in sides:
                pb = Rd * PP_W + PP_SLOT[k] + psel * PP_MIR
                c0 = c
                while c0 > 2:
                    fl, ce = c0 // 2, c0 - c0 // 2
                    nc.gpsimd.tensor_tensor(
                        ap(pp[:], pb, [[PP_W, Rp], [1, fl]]),
                        ap(pp[:], pb, [[PP_W, Rp], [1, fl]]),
                        ap(pp[:], pb + ce, [[PP_W, Rp], [1, fl]]),
                        AluOpType.mult)
                    c0 = ce
                nc.gpsimd.tensor_tensor(
                    ap(lpt[:], Rd * 19 + kk, [[19, Rp], [1, 1]]),
                    ap(pp[:], pb, [[PP_W, Rp], [1, 1]]),
                    ap(pp[:], pb + 1, [[PP_W, Rp], [1, 1]]),
                    AluOpType.mult)

    # normalize: y = (1 - P) / (19 + 1e-9 - sum P); sum P as Pool add-tree
    st_ = sm.tile([P, R], F32, tag="S")
    sacc = sm.tile([P, R, 9], F32, tag="sacc")
    nc.gpsimd.tensor_tensor(sacc[:], lpt[:, :, 0:9], lpt[:, :, 10:19],
                            AluOpType.add)
    nc.gpsimd.tensor_tensor(sacc[:, :, 0:4], sacc[:, :, 0:4],
                            sacc[:, :, 5:9], AluOpType.add)
    nc.gpsimd.tensor_tensor(sacc[:, :, 0:2], sacc[:, :, 0:2],
                            sacc[:, :, 2:4], AluOpType.add)
    nc.gpsimd.tensor_tensor(sacc[:, :, 0:1], sacc[:, :, 0:1],
                            sacc[:, :, 1:2], AluOpType.add)
    nc.gpsimd.tensor_tensor(sacc[:, :, 4:5], sacc[:, :, 4:5],
                            lpt[:, :, 9:10], AluOpType.add)
    nc.gpsimd.tensor_tensor(st_[:].unsqueeze(2), sacc[:, :, 0:1],
                            sacc[:, :, 4:5], AluOpType.add)
    nc.gpsimd.tensor_scalar(st_[:], st_[:], -1.0, 19.0 + 1e-9,
                            AluOpType.mult, AluOpType.add)
    rt = sm.tile([P, R], F32, tag="r")
    nc.vector.reciprocal(rt[:], st_[:])
    nc.scalar.activation(
        lpt[:].rearrange("p r k -> p (r k)"),
        lpt[:].rearrange("p r k -> p (r k)"),
        mybir.ActivationFunctionType.Copy, bias=1.0, scale=-1.0)
    nc.gpsimd.tensor_tensor(
        lpt[:], lpt[:],
        rt[:].unsqueeze(2).broadcast_to((P, R, 19)),
        AluOpType.mult)
    nc.sync.dma_start(yv, lpt[:].rearrange("p r k -> p (r k)"))


def _build_core(rows_total, sched, rd_map):
    import concourse.mybir as mybir
    from concourse.bacc import Bacc
    from concourse.tile import TileContext

    F32 = mybir.dt.float32

    nc = Bacc()
    p1d = nc.dram_tensor("p1", [rows_total, 10], F32, kind="ExternalInput")
    p2d = nc.dram_tensor("p2", [rows_total, 10], F32, kind="ExternalInput")
    v1d = nc.dram_tensor("v1b", [120, 120], F32, kind="ExternalInput")
    v2d = nc.dram_tensor("v2b", [120, 120], F32, kind="ExternalInput")
    idd = nc.dram_tensor("ident", [128, 128], F32, kind="ExternalInput")
    yd = nc.dram_tensor("y", [rows_total, 19], F32, kind="ExternalOutput")

    with TileContext(nc) as tc:
        with (
            tc.tile_pool(name="const", bufs=1) as cpool,
            tc.tile_pool(name="io", bufs=3) as io,
            tc.tile_pool(name="pt", bufs=2) as ptp,
            tc.tile_pool(name="abf", bufs=3) as abf,
            tc.tile_pool(name="abb", bufs=3) as abb,
            tc.tile_pool(name="pps", bufs=2) as pps,
            tc.tile_pool(name="lpt", bufs=3) as lptp,
            tc.tile_pool(name="e4", bufs=2) as e4p,
            tc.tile_pool(name="sm", bufs=2) as sm,
            tc.tile_pool(name="tp", bufs=2, space="PSUM") as tpp,
            tc.tile_pool(name="mm", bufs=2, space="PSUM") as mmp,
        ):
            v1t = cpool.tile([120, 120], F32)
            v2t = cpool.tile([120, 120], F32)
            idt = cpool.tile([128, 128], F32)
            nc.sync.dma_start(idt[:], idd[:])
            nc.sync.dma_start(v1t[:], v1d[:])
            nc.sync.dma_start(v2t[:], v2d[:])
            pools = (io, ptp, abf, abb, pps, lptp, e4p, sm, tpp, mmp)
            consts = (v1t, v2t, idt)

            n = len(sched)
            loads, fronts = {}, {}
            for i in range(min(2, n)):
                loads[i] = _emit_loads(nc, io, p1d, p2d, *sched[i])
            if n:
                fronts[0] = _emit_front(nc, pools, consts, sched[0][1],
                                        loads.pop(0))
            for i in range(n):
                if i + 2 < n:
                    loads[i + 2] = _emit_loads(nc, io, p1d, p2d,
                                               *sched[i + 2])
                if i + 1 < n:
                    fronts[i + 1] = _emit_front(nc, pools, consts,
                                                sched[i + 1][1],
                                                loads.pop(i + 1))
                row0, R = sched[i]
                nrows = P * R
                yv = yd[row0:row0 + nrows, :].rearrange(
                    "(p r) k -> p (r k)", p=P)
                _emit_back(nc, pools, fronts.pop(i), yv, R, rd_map[R])

    nc.finalize()
    return nc


RD_MAP = {128: 31, 108: 24, 96: 21, 64: 14, 48: 10, 36: 8,
          32: 7, 24: 5, 16: 3, 12: 2, 8: 2}


def _build_nc(nt=NT, reps=1):
    sched = [s for _ in range(reps) for s in _schedule(nt)]
    return _build_core(RPC, sched, RD_MAP)


def _host_consts(W1, W2):
    def mmn(W):
        W = W.astype(np.float32)
        lo = W.min(1, keepdims=True)
        hi = W.max(1, keepdims=True)
        return (W - lo) / (hi - lo + np.float32(1e-8))

    eye12 = np.eye(12, dtype=np.float32)
    v1b = np.kron(eye12, (np.float32(1.0) - mmn(W1))).astype(np.float32)
    v2b = np.kron(eye12, (np.float32(1.0) - mmn(W2))).astype(np.float32)
    ident = np.eye(128, dtype=np.float32)
    return v1b, v2b, ident


def kernel(p1, p2, W1, W2, mask=None, **_unused):
    from concourse.bass_utils import run_bass_kernel_spmd

    if 'nc' not in _CACHED:
        _CACHED['nc'] = _build_nc()
    nc = _CACHED['nc']

    v1b, v2b, ident = _host_consts(W1, W2)
    p1 = np.ascontiguousarray(p1, dtype=np.float32)
    p2 = np.ascontiguousarray(p2, dtype=np.float32)

    in_maps = []
    for c in range(N_CORES):
        sl = slice(c * RPC, (c + 1) * RPC)
        in_maps.append({
            "p1": p1[sl], "p2": p2[sl],
            "v1b": v1b, "v2b": v2b, "ident": ident,
        })
    res = run_bass_kernel_spmd(nc, in_maps, list(range(N_CORES)))
    out = np.concatenate([res.results[c]["y"] for c in range(N_CORES)], axis=0)
    return out.astype(np.float32)


def _numpy_ref(p1, p2, W1, W2):
    def mmn(W):
        lo = W.min(1, keepdims=True); hi = W.max(1, keepdims=True)
        return (W - lo) / (hi - lo + np.float32(1e-8))
    A = p1 @ (1.0 - mmn(W1))
    Bv = p2 @ (1.0 - mmn(W2))
    st = np.maximum(A[:, :, None], Bv[:, None, :])
    Pk = np.ones((p1.shape[0], 19), np.float64)
    for i in range(10):
        for j in range(10):
            Pk[:, i + j] *= st[:, i, j]
    y = 1.0 - Pk
    return (y / (y.sum(1, keepdims=True) + 1e-9)).astype(np.float32)


def _simcheck():
    """CoreSim-exec validation on a small schedule."""
    from concourse.bass_interp import CoreSim
    rows = P * 16
    nc = _build_core(rows, [(0, 8), (P * 8, 8)], RD_MAP)
    rng = np.random.default_rng(1)
    p1 = rng.random((rows, 10), dtype=np.float32)
    p1 /= p1.sum(1, keepdims=True)
    p2 = rng.random((rows, 10), dtype=np.float32)
    p2 /= p2.sum(1, keepdims=True)
    W1 = rng.random((10, 10), dtype=np.float32)
    W2 = rng.random((10, 10), dtype=np.float32)
    v1b, v2b, ident = _host_consts(W1, W2)

    vals = {"p1": p1, "p2": p2, "v1b": v1b, "v2b": v2b, "ident": ident}
    bufs = {}
    for alloc in nc.m.functions[0].allocations:
        if hasattr(alloc, 'memorylocations') and alloc.memorylocations:
            for mem in alloc.memorylocations:
                if mem.name in vals:
                    a = np.ascontiguousarray(vals[mem.name])
                    bufs[mem.name] = a.view(np.uint8).reshape(tuple(mem.dims))
                elif mem.name == "y":
                    bufs["y"] = np.zeros(tuple(mem.dims), np.uint8)
    sim = CoreSim(nc, preallocated_bufs=bufs)
    sim.simulate()
    y = bufs["y"].view(np.float32).reshape(rows, 19)
    ref = _numpy_ref(p1, p2, W1, W2)
    rel = np.abs(y - ref) / np.maximum(np.abs(ref), 1e-12)
    print(f"simcheck: rel err max {rel.max():.3e}  sim time {sim.time:.0f} ns")
    assert rel.max() < 2e-2, "simcheck FAILED"
    print("simcheck PASSED")


if __name__ == "__main__":
    if "--simcheck" in sys.argv:
        _simcheck()
    else:
        rng = np.random.default_rng(0)
        p1 = rng.random((B, 10), dtype=np.float32)
        p1 /= p1.sum(1, keepdims=True)
        p2 = rng.random((B, 10), dtype=np.float32)
        p2 /= p2.sum(1, keepdims=True)
        W1 = rng.random((10, 10), dtype=np.float32)
        W2 = rng.random((10, 10), dtype=np.float32)
        y = kernel(p1, p2, W1, W2)
        print("kernel ran, y shape", y.shape, "sum", float(y.sum()))


# revision 39
# speedup vs baseline: 2.6519x; 1.0001x over previous
"""Trainium2 Bass kernel for nn_BaconAdditionReasoner (segment_reduce).

Math (per row b of 1M):
  a = p1 @ minmax(W1); b = p2 @ minmax(W2)           # [10] each
  s_ij = min(a_i, b_j); one_minus = 1 - clip(s)       # [10,10]
  y_k  = 1 - prod_{i+j=k} one_minus_ij                # 19 anti-diag bins
  y    = y / (sum_k y_k + 1e-9)

Kernel formulation (linear space, no Ln/Exp round-trip):
  A = p1 @ (1 - minmax(W1)); B = p2 @ (1 - minmax(W2))   # = one_minus rows
  one_minus_ij = max(A_i, B_j)   [clip never fires: A,B in (0.11, 0.84)]
  P_k = prod over anti-diagonal; y = (1 - P) / (19 + 1e-9 - sum P)

Engine split per tile (R rows/partition, rows on 128 partitions):
  PE    : 12-row-packed transposes + matmuls vs kron(I_12, V)
  ACT   : PSUM->SBUF copies (4-group-batched transpose spill and A|B
          copies), u = 1 - P
  DVE   : bf16 pair-packed max over mirror-bin pairs for bins 2..16
          (all strides +-1 -> 2x_1p double rate); fp32 edge bins
          0/1/17/18 (error amplification P/(1-P) is up to 4.1 there,
          <=0.54 for mid bins, so bf16 is only safe for the latter);
          bf16 multiply-trees for rows [0, Rd); reciprocal
  Pool  : multiply-trees for rows [Rd, R) via per-bin 2-dim APs (Pool
          ucode implements add/sub/mult/copy only -- no max/min/PSUM),
          the 19-bin add-tree for sum P, and y = u * r

Tiles are emitted software-pipelined (frontend of tile i+1 before the
backend of tile i, input DMAs two tiles ahead) so the in-order engine
queues interleave ready work instead of stalling on cross-engine waits.

Sharding: pure data parallel over 8 cores, 131072 rows each.
"""
import sys

if '/opt/trn_rl_repo' not in sys.path:
    sys.path.insert(0, '/opt/trn_rl_repo')

import numpy as np

B = 1048576
N_CORES = 8
RPC = B // N_CORES          # 131072 rows per core
P = 128                     # partitions
NT = 16                     # work units of 8192 rows (bench scaling)

# Uniform-slot layout for bins (k, 18-k), k=1..8 (cnt c=k+1), plus bin 9:
# bin k lives at slot (k-1)*10, its mirror at +90, bin 9 at 80. The
# constant slot/mirror strides let multiply-tree levels merge across
# adjacent pairs (equal fl; the ce offsets line up as stride-11 APs).
# Bins 1/17 ride the bf16 path: their amplification P/(1-P) is <= 0.98 so
# the ~0.8% worst-case bf16 error stays well under the 2e-2 gate; only
# bins 0/18 (ratio up to 4.1) need fp32.
MID_KS = [1, 2, 3, 4, 5, 6, 7, 8]
PP_SLOT = {k: (k - 1) * 10 for k in MID_KS}
PP_SLOT[9] = 80
PP_MIR = 90
PP_W = 180


def _groups_for(r):
    g = [12] * (r // 12)
    if r % 12:
        g.append(r % 12)
    return g


def _batches_for(r):
    """<=4 transpose groups (<=480 PSUM columns) per PSUM tile / copy."""
    gs = _groups_for(r)
    out, cur = [], []
    for g in gs:
        cur.append(g)
        if len(cur) == 4:
            out.append(cur)
            cur = []
    if cur:
        out.append(cur)
    return out


def _schedule(nt):
    rows = nt * P * 64
    rpp = rows // P
    # ramp tile sizes up at the start and down at the end: shortens the
    # pipeline fill (engines wait on tile 0's serial frontend) and the
    # drain (the last tile's serial backend tail)
    ramp_up, ramp_dn = [12, 12, 24, 48, 96], [96, 48, 36, 12]
    out, row0 = [], 0
    if rpp >= sum(ramp_up) + sum(ramp_dn) + 128:
        mid = rpp - sum(ramp_up) - sum(ramp_dn)
        plan = list(ramp_up) + [128] * (mid // 128)
        if mid % 128:
            plan.append(mid % 128)
        plan += ramp_dn
    else:
        plan = []
        left = rpp
        while left > 0:
            r = min(128, left)
            plan.append(r); left -= r
    for r in plan:
        out.append((row0, r)); row0 += P * r
    assert row0 == rows
    return out

_CACHED = {}


def _mkap(br, tile_ap, off, dims):
    return br.AP(tensor=tile_ap.tensor, offset=tile_ap.offset + off,
                 ap=[list(tile_ap.ap[0])] + [list(d) for d in dims])


def _emit_loads(nc, io, p1d, p2d, row0, R):
    import concourse.mybir as mybir
    F32 = mybir.dt.float32
    nrows = P * R
    p1v = p1d[row0:row0 + nrows, :].rearrange("(p r) c -> p (r c)", p=P)
    p2v = p2d[row0:row0 + nrows, :].rearrange("(p r) c -> p (r c)", p=P)
    p1t = io.tile([P, R * 10], F32, tag="p1t")
    p2t = io.tile([P, R * 10], F32, tag="p2t")
    nc.sync.dma_start(p1t[:], p1v)
    nc.sync.dma_start(p2t[:], p2v)
    return p1t, p2t


def _emit_front(nc, pools, consts, R, loaded):
    """Transposes, matmuls, PSUM->SBUF copies.

    Produces abb [P, R, 20] bf16 (A | B rows for the pair-packed max) and
    abe [P, R, 4] fp32 (A0 A9 B0 B9, for the fp32 edge bins 0/18)."""
    import bass_rust as _br
    import concourse.mybir as mybir

    F32 = mybir.dt.float32
    BF16 = mybir.dt.bfloat16
    io, ptp, abf, abb, pps, lptp, e4p, sm, tpp, mmp = pools
    v1t, v2t, idt = consts
    p1t, p2t = loaded

    abb_t = abb.tile([P, R, 20], BF16, tag="abb")
    abe = abf.tile([P, R, 4], F32, tag="abe")
    r0 = 0
    for batch in _batches_for(R):
        brows = sum(batch)
        bcols = brows * 10
        mma = mmp.tile([P, 480], F32, tag="mma")
        mmb = mmp.tile([P, 480], F32, tag="mmb")
        tpa = tpp.tile([120, 512], F32, tag="tpa")
        tpb = tpp.tile([120, 512], F32, tag="tpb")
        pta = ptp.tile([120, 512], F32, tag="pta")
        ptb = ptp.tile([120, 512], F32, tag="ptb")
        for src, vt, mm, tp, pt in ((p1t, v1t, mma, tpa, pta),
                                    (p2t, v2t, mmb, tpb, ptb)):
            off = 0
            full_k = 0
            for gi, gs in enumerate(batch):
                K = gs * 10
                nc.tensor.transpose(
                    tp[0:K, gi * 128:(gi + 1) * 128],
                    src[:, (r0 * 10 + off):(r0 * 10 + off + K)], idt[:])
                off += K
                if K == 120:
                    full_k += 1
            # batched PSUM->SBUF copy: full 120-row groups in one shot,
            # the ragged tail group (if any) separately
            if full_k:
                nc.scalar.copy(pt[:, 0:full_k * 128], tp[:, 0:full_k * 128])
            if full_k < len(batch):
                K = batch[-1] * 10
                nc.scalar.copy(pt[0:K, full_k * 128:full_k * 128 + 128],
                               tp[0:K, full_k * 128:full_k * 128 + 128])
            off = 0
            for gi, gs in enumerate(batch):
                K = gs * 10
                nc.tensor.matmul(mm[:, off:off + K],
                                 pt[0:K, gi * 128:(gi + 1) * 128],
                                 vt[0:K, 0:K], start=True, stop=True)
                off += K
        for mm, col in ((mma, 0), (mmb, 10)):
            dst = _mkap(_br, abb_t[:], r0 * 20 + col, [[20, brows], [1, 10]])
            nc.scalar.copy(
                dst, mm[:, 0:bcols].rearrange("p (r c) -> p r c", c=10))
        for mm, col in ((mma, 0), (mmb, 2)):
            dst = _mkap(_br, abe[:], r0 * 4 + col, [[4, brows], [1, 2]])
            src = _mkap(_br, mm[:, 0:bcols], 0, [[10, brows], [9, 2]])
            nc.scalar.copy(dst, src)
        r0 += brows
    return abb_t, abe


def _emit_back(nc, pools, front, yv, R, Rd):
    """Max, multiply-trees, normalize, store."""
    import bass_rust as _br
    import concourse.mybir as mybir
    from concourse.mybir import AluOpType

    F32 = mybir.dt.float32
    BF16 = mybir.dt.bfloat16
    io, ptp, abf, abb, pps, lptp, e4p, sm, tpp, mmp = pools
    abb_t, abe = front
    Rp = R - Rd

    def ap(tile_ap, off, dims):
        return _mkap(_br, tile_ap, off, dims)

    lpt = lptp.tile([P, R, 19], F32, tag="lpt")
    pp = pps.tile([P, R, PP_W], BF16, tag="pp")
    A, Bo = 0, 10
    for k in MID_KS:
        c = k + 1
        nc.vector.tensor_tensor(
            ap(pp[:], PP_SLOT[k], [[PP_W, R], [PP_MIR, 2], [1, c]]),
            ap(abb_t[:], A, [[20, R], [9 - k, 2], [1, c]]),
            ap(abb_t[:], Bo + k, [[20, R], [9 - k, 2], [-1, c]]),
            AluOpType.max)
    nc.vector.tensor_tensor(
        ap(pp[:], PP_SLOT[9], [[PP_W, R], [1, 10]]),
        ap(abb_t[:], A, [[20, R], [1, 10]]),
        ap(abb_t[:], Bo + 9, [[20, R], [-1, 10]]),
        AluOpType.max)
    # edge bins 0/18 in fp32 (abe cols: A0 A9 B0 B9)
    nc.vector.tensor_tensor(
        ap(lpt[:], 0, [[19, R], [18, 2]]),
        ap(abe[:], 0, [[4, R], [1, 2]]),
        ap(abe[:], 2, [[4, R], [1, 2]]),
        AluOpType.max)
    # multiply-trees. DVE rows [0, Rd): levels merged across adjacent
    # pairs (uniform slot layout; in1 offsets align at stride 11 where ce
    # increments). Pool rows [Rd, R): per-bin 2-dim APs (Pool ucode has
    # mult but not max/min). Final level writes fp32 into lpt.
    if Rd > 0:
        M2 = [[PP_MIR, 2]]
        mul = AluOpType.mult
        def dv(o_off, o_dims, i_off, i_dims):
            nc.vector.tensor_tensor(ap(pp[:], o_off, [[PP_W, Rd]] + o_dims),
                                    ap(pp[:], o_off, [[PP_W, Rd]] + o_dims),
                                    ap(pp[:], i_off, [[PP_W, Rd]] + i_dims),
                                    mul)
        # L1
        dv(10, M2 + [[1, 1]], 12, M2 + [[1, 1]])                      # c3
        dv(20, M2 + [[10, 2], [1, 2]], 22, M2 + [[11, 2], [1, 2]])    # c4,c5
        dv(40, M2 + [[10, 2], [1, 3]], 43, M2 + [[11, 2], [1, 3]])    # c6,c7
        dv(60, M2 + [[10, 2], [1, 4]], 64, M2 + [[11, 2], [1, 4]])    # c8,c9
        dv(80, [[1, 5]], 85, [[1, 5]])                                # c10
        # L2
        dv(30, M2 + [[10, 2], [1, 1]], 32, M2 + [[10, 2], [1, 1]])    # c5,c6
        dv(50, M2 + [[10, 2], [1, 2]], 52, M2 + [[10, 2], [1, 2]])    # c7,c8
        dv(70, M2 + [[1, 2]], 73, M2 + [[1, 2]])                      # c9
        dv(80, [[1, 2]], 83, [[1, 2]])                                # c10
        # L3
        dv(70, M2 + [[1, 1]], 72, M2 + [[1, 1]])                      # c9
        dv(80, [[1, 1]], 82, [[1, 1]])                                # c10
        # finals -> lpt (fp32)
        for k in MID_KS:
            nc.vector.tensor_tensor(
                ap(lpt[:], k, [[19, Rd], [18 - 2 * k, 2], [1, 1]]),
                ap(pp[:], PP_SLOT[k], [[PP_W, Rd]] + M2 + [[1, 1]]),
                ap(pp[:], PP_SLOT[k] + 1, [[PP_W, Rd]] + M2 + [[1, 1]]),
                mul)
        nc.vector.tensor_tensor(
            ap(lpt[:], 9, [[19, Rd], [1, 1]]),
            ap(pp[:], 80, [[PP_W, Rd], [1, 1]]),
            ap(pp[:], 81, [[PP_W, Rd], [1, 1]]), mul)
    if Rp > 0:
        for k in MID_KS + [9]:
            c = k + 1 if k != 9 else 10
            sides = (((0, k), (1, 18 - k)) if k != 9 else ((0, 9),))
            for psel, kk in sides:
                pb = Rd * PP_W + PP_SLOT[k] + psel * PP_MIR
                c0 = c
                while c0 > 2:
                    fl, ce = c0 // 2, c0 - c0 // 2
                    nc.gpsimd.tensor_tensor(
                        ap(pp[:], pb, [[PP_W, Rp], [1, fl]]),
                        ap(pp[:], pb, [[PP_W, Rp], [1, fl]]),
                        ap(pp[:], pb + ce, [[PP_W, Rp], [1, fl]]),
                        AluOpType.mult)
                    c0 = ce
                nc.gpsimd.tensor_tensor(
                    ap(lpt[:], Rd * 19 + kk, [[19, Rp], [1, 1]]),
                    ap(pp[:], pb, [[PP_W, Rp], [1, 1]]),
                    ap(pp[:], pb + 1, [[PP_W, Rp], [1, 1]]),
                    AluOpType.mult)

    # normalize: y = (1 - P) / (19 + 1e-9 - sum P); sum P as Pool add-tree
    st_ = sm.tile([P, R], F32, tag="S")
    sacc = sm.tile([P, R, 9], F32, tag="sacc")
    nc.gpsimd.tensor_tensor(sacc[:], lpt[:, :, 0:9], lpt[:, :, 10:19],
                            AluOpType.add)
    nc.gpsimd.tensor_tensor(sacc[:, :, 0:4], sacc[:, :, 0:4],
                            sacc[:, :, 5:9], AluOpType.add)
    nc.gpsimd.tensor_tensor(sacc[:, :, 0:2], sacc[:, :, 0:2],
                            sacc[:, :, 2:4], AluOpType.add)
    nc.gpsimd.tensor_tensor(sacc[:, :, 0:1], sacc[:, :, 0:1],
                            sacc[:, :, 1:2], AluOpType.add)
    nc.gpsimd.tensor_tensor(sacc[:, :, 4:5], sacc[:, :, 4:5],
                            lpt[:, :, 9:10], AluOpType.add)
    nc.gpsimd.tensor_tensor(st_[:].unsqueeze(2), sacc[:, :, 0:1],
                            sacc[:, :, 4:5], AluOpType.add)
    nc.gpsimd.tensor_scalar(st_[:], st_[:], -1.0, 19.0 + 1e-9,
                            AluOpType.mult, AluOpType.add)
    rt = sm.tile([P, R], F32, tag="r")
    nc.vector.reciprocal(rt[:], st_[:])
    nc.scalar.activation(
        lpt[:].rearrange("p r k -> p (r k)"),
        lpt[:].rearrange("p r k -> p (r k)"),
        mybir.ActivationFunctionType.Copy, bias=1.0, scale=-1.0)
    nc.gpsimd.tensor_tensor(
        lpt[:], lpt[:],
        rt[:].unsqueeze(2).broadcast_to((P, R, 19)),
        AluOpType.mult)
    nc.sync.dma_start(yv, lpt[:].rearrange("p r k -> p (r k)"))


def _build_core(rows_total, sched, rd_map):
    import concourse.mybir as mybir
    from concourse.bacc import Bacc
    from concourse.tile import TileContext

    F32 = mybir.dt.float32

    nc = Bacc()
    p1d = nc.dram_tensor("p1", [rows_total, 10], F32, kind="ExternalInput")
    p2d = nc.dram_tensor("p2", [rows_total, 10], F32, kind="ExternalInput")
    v1d = nc.dram_tensor("v1b", [120, 120], F32, kind="ExternalInput")
    v2d = nc.dram_tensor("v2b", [120, 120], F32, kind="ExternalInput")
    idd = nc.dram_tensor("ident", [128, 128], F32, kind="ExternalInput")
    yd = nc.dram_tensor("y", [rows_total, 19], F32, kind="ExternalOutput")

    with TileContext(nc) as tc:
        with (
            tc.tile_pool(name="const", bufs=1) as cpool,
            tc.tile_pool(name="io", bufs=3) as io,
            tc.tile_pool(name="pt", bufs=2) as ptp,
            tc.tile_pool(name="abf", bufs=3) as abf,
            tc.tile_pool(name="abb", bufs=3) as abb,
            tc.tile_pool(name="pps", bufs=2) as pps,
            tc.tile_pool(name="lpt", bufs=3) as lptp,
            tc.tile_pool(name="e4", bufs=2) as e4p,
            tc.tile_pool(name="sm", bufs=2) as sm,
            tc.tile_pool(name="tp", bufs=2, space="PSUM") as tpp,
            tc.tile_pool(name="mm", bufs=2, space="PSUM") as mmp,
        ):
            v1t = cpool.tile([120, 120], F32)
            v2t = cpool.tile([120, 120], F32)
            idt = cpool.tile([128, 128], F32)
            nc.sync.dma_start(idt[:], idd[:])
            nc.sync.dma_start(v1t[:], v1d[:])
            nc.sync.dma_start(v2t[:], v2d[:])
            pools = (io, ptp, abf, abb, pps, lptp, e4p, sm, tpp, mmp)
            consts = (v1t, v2t, idt)

            n = len(sched)
            loads, fronts = {}, {}
            for i in range(min(2, n)):
                loads[i] = _emit_loads(nc, io, p1d, p2d, *sched[i])
            if n:
                fronts[0] = _emit_front(nc, pools, consts, sched[0][1],
                                        loads.pop(0))
            for i in range(n):
                if i + 2 < n:
                    loads[i + 2] = _emit_loads(nc, io, p1d, p2d,
                                               *sched[i + 2])
                if i + 1 < n:
                    fronts[i + 1] = _emit_front(nc, pools, consts,
                                                sched[i + 1][1],
                                                loads.pop(i + 1))
                row0, R = sched[i]
                nrows = P * R
                yv = yd[row0:row0 + nrows, :].rearrange(
                    "(p r) k -> p (r k)", p=P)
                _emit_back(nc, pools, fronts.pop(i), yv, R, rd_map[R])

    nc.finalize()
    return nc


RD_MAP = {128: 31, 108: 24, 96: 22, 64: 14, 48: 10, 36: 8,
          32: 7, 24: 5, 16: 3, 12: 2, 8: 2}


def _build_nc(nt=NT, reps=1):
    sched = [s for _ in range(reps) for s in _schedule(nt)]
    return _build_core(RPC, sched, RD_MAP)


def _host_consts(W1, W2):
    def mmn(W):
        W = W.astype(np.float32)
        lo = W.min(1, keepdims=True)
        hi = W.max(1, keepdims=True)
        return (W - lo) / (hi - lo + np.float32(1e-8))

    eye12 = np.eye(12, dtype=np.float32)
    v1b = np.kron(eye12, (np.float32(1.0) - mmn(W1))).astype(np.float32)
    v2b = np.kron(eye12, (np.float32(1.0) - mmn(W2))).astype(np.float32)
    ident = np.eye(128, dtype=np.float32)
    return v1b, v2b, ident


def kernel(p1, p2, W1, W2, mask=None, **_unused):
    from concourse.bass_utils import run_bass_kernel_spmd

    if 'nc' not in _CACHED:
        _CACHED['nc'] = _build_nc()
    nc = _CACHED['nc']

    v1b, v2b, ident = _host_consts(W1, W2)
    p1 = np.ascontiguousarray(p1, dtype=np.float32)
    p2 = np.ascontiguousarray(p2, dtype=np.float32)

    in_maps = []
    for c in range(N_CORES):
        sl = slice(c * RPC, (c + 1) * RPC)
        in_maps.append({
            "p1": p1[sl], "p2": p2[sl],
            "v1b": v1b, "v2b": v2b, "ident": ident,
        })
    res = run_bass_kernel_spmd(nc, in_maps, list(range(N_CORES)))
    out = np.concatenate([res.results[c]["y"] for c in range(N_CORES)], axis=0)
    return out.astype(np.float32)


def _numpy_ref(p1, p2, W1, W2):
    def mmn(W):
        lo = W.min(1, keepdims=True); hi = W.max(1, keepdims=True)
        return (W - lo) / (hi - lo + np.float32(1e-8))
    A = p1 @ (1.0 - mmn(W1))
    Bv = p2 @ (1.0 - mmn(W2))
    st = np.maximum(A[:, :, None], Bv[:, None, :])
    Pk = np.ones((p1.shape[0], 19), np.float64)
    for i in range(10):
        for j in range(10):
            Pk[:, i + j] *= st[:, i, j]
    y = 1.0 - Pk
    return (y / (y.sum(1, keepdims=True) + 1e-9)).astype(np.float32)


def _simcheck():
    """CoreSim-exec validation on a small schedule."""
    from concourse.bass_interp import CoreSim
    rows = P * 16
    nc = _build_core(rows, [(0, 8), (P * 8, 8)], RD_MAP)
    rng = np.random.default_rng(1)
    p1 = rng.random((rows, 10), dtype=np.float32)
    p1 /= p1.sum(1, keepdims=True)
    p2 = rng.random((rows, 10), dtype=np.float32)
    p2 /= p2.sum(1, keepdims=True)
    W1 = rng.random((10, 10), dtype=np.float32)
    W2 = rng.random((10, 10), dtype=np.float32)
    v1b, v2b, ident = _host_consts(W1, W2)

    vals = {"p1": p1, "p2": p2, "v1b": v1b, "v2b": v2b, "ident": ident}
    bufs = {}
    for alloc in nc.m.functions[0].allocations:
        if hasattr(alloc, 'memorylocations') and alloc.memorylocations:
            for mem in alloc.memorylocations:
                if mem.name in vals:
                    a = np.ascontiguousarray(vals[mem.name])
                    bufs[mem.name] = a.view(np.uint8).reshape(tuple(mem.dims))
                elif mem.name == "y":
                    bufs["y"] = np.zeros(tuple(mem.dims), np.uint8)
    sim = CoreSim(nc, preallocated_bufs=bufs)
    sim.simulate()
    y = bufs["y"].view(np.float32).reshape(rows, 19)
    ref = _numpy_ref(p1, p2, W1, W2)
    rel = np.abs(y - ref) / np.maximum(np.abs(ref), 1e-12)
    print(f"simcheck: rel err max {rel.max():.3e}  sim time {sim.time:.0f} ns")
    assert rel.max() < 2e-2, "simcheck FAILED"
    print("simcheck PASSED")


if __name__ == "__main__":
    if "--simcheck" in sys.argv:
        _simcheck()
    else:
        rng = np.random.default_rng(0)
        p1 = rng.random((B, 10), dtype=np.float32)
        p1 /= p1.sum(1, keepdims=True)
        p2 = rng.random((B, 10), dtype=np.float32)
        p2 /= p2.sum(1, keepdims=True)
        W1 = rng.random((10, 10), dtype=np.float32)
        W2 = rng.random((10, 10), dtype=np.float32)
        y = kernel(p1, p2, W1, W2)
        print("kernel ran, y shape", y.shape, "sum", float(y.sum()))
